# revision 1
# baseline (speedup 1.0000x reference)
"""GQA (32 q heads / 8 kv heads, RoPE, causal) Trainium2 Bass kernel.

Sharding: tensor-parallel over kv heads — core c owns kv head c and q heads
4c..4c+3 for both batches. Each core computes a partial o-projection
(its 256 attn channels x Wo columns) and the host sums the 8 partials.

Device-side structure (per core, per batch):
  * Fused QKV projection: one accumulation chain per 128-token tile produces
    [t, 384] = [4 q heads | k head | v head] with d contracted on partitions
    (host passes x pre-transposed).  float32r matmuls (1 cycle/row).
  * RoPE applied in token-partition layout with stride-2 free-dim APs
    (interleaved even/odd pairs), 6 DVE ops per tile covering all 5 heads.
  * Q/K transposed per-head via TensorE into [dh, t] (f32r), V kept natural
    [t, dh] with a ones column appended.
  * Scores computed transposed [keys, queries]; exp on ACT (no max needed:
    |scores| small by construction); causal diagonal masked by DVE multiply.
  * attn.V matmul gives attnT [dh, i] plus the softmax denominator for free
    (ones row of V); normalization via reciprocal + PE broadcast + DVE mul
    writes attnT directly into the o-projection's stationary layout [c, t].
"""

import numpy as np
from contextlib import ExitStack

import concourse.bass as bass
from concourse import bacc
import concourse.mybir as mybir
import concourse.tile as tile
from concourse.bass_utils import run_bass_kernel_spmd

B, S, D = 2, 2048, 2048
DH = 64            # head dim
G = 4              # q heads per core (= per kv head)
NCORES = 8
TT = 512           # attention i-tile
NTT = S // TT      # 4
KC = D // 128      # 16 contraction chunks
NJC = S // 128     # 16 token/key chunks of 128
F32 = mybir.dt.float32
F32R = mybir.dt.float32r
ROPE_BASE = 10000.0

_cached = {}


def build_nc():
    nc = bacc.Bacc("TRN2", target_bir_lowering=False, debug=False)
    xt = nc.declare_dram_parameter("xt", [B, D, S], F32, isOutput=False)
    wall = nc.declare_dram_parameter("wall", [D, 384], F32, isOutput=False)
    wot = nc.declare_dram_parameter("wot", [256, D], F32, isOutput=False)
    cosr = nc.declare_dram_parameter("cosr", [S, 160], F32, isOutput=False)
    sinr = nc.declare_dram_parameter("sinr", [S, 160], F32, isOutput=False)
    cmask = nc.declare_dram_parameter("cmask", [4, 128, TT], F32, isOutput=False)
    ident = nc.declare_dram_parameter("ident", [128, 128], F32, isOutput=False)
    o = nc.declare_dram_parameter("o", [B, S, D], F32, isOutput=True)

    EXP = mybir.ActivationFunctionType.Exp

    with tile.TileContext(nc) as tc, ExitStack() as ctx:
        wpool = ctx.enter_context(tc.tile_pool(name="weights", bufs=1))
        per_b = ctx.enter_context(tc.tile_pool(name="per_b", bufs=1))
        xpool = ctx.enter_context(tc.tile_pool(name="xstream", bufs=12))
        qkvpool = ctx.enter_context(tc.tile_pool(name="qkv", bufs=3))
        epool = ctx.enter_context(tc.tile_pool(name="exp", bufs=6))
        rpool = ctx.enter_context(tc.tile_pool(name="rope", bufs=2))
        opool = ctx.enter_context(tc.tile_pool(name="out", bufs=4))
        spool = ctx.enter_context(tc.tile_pool(name="small", bufs=4))
        pp_proj = ctx.enter_context(tc.tile_pool(name="pproj", bufs=1, space="PSUM"))
        pp_att = ctx.enter_context(tc.tile_pool(name="patt", bufs=2, space="PSUM"))
        pp_av = ctx.enter_context(tc.tile_pool(name="pav", bufs=1, space="PSUM"))
        pp_misc = ctx.enter_context(tc.tile_pool(name="pmisc", bufs=1, space="PSUM"))

        # ---- persistent weights/tables ----
        wall_sb = wpool.tile([128, KC, 384], F32R, tag="wall")
        wot_sb = wpool.tile([128, 2, D], F32R, tag="wot")
        cos_sb = wpool.tile([128, NJC, 160], F32, tag="cos")
        sin_sb = wpool.tile([128, NJC, 160], F32, tag="sin")
        mask_sb = wpool.tile([128, 4, TT], F32R, tag="mask")
        ident_sb = wpool.tile([128, 128], F32, tag="ident")
        ones_sb = wpool.tile([1, 64], F32R, tag="ones")
        for k in range(KC):
            nc.sync.dma_start(wall_sb[:, k, :],
                              wall[k * 128:(k + 1) * 128, :].bitcast(F32R))
        for cc in range(2):
            nc.sync.dma_start(wot_sb[:, cc, :],
                              wot[cc * 128:(cc + 1) * 128, :].bitcast(F32R))
        for j in range(NJC):
            nc.sync.dma_start(cos_sb[:, j, :], cosr[j * 128:(j + 1) * 128, :])
            nc.sync.dma_start(sin_sb[:, j, :], sinr[j * 128:(j + 1) * 128, :])
        for m in range(4):
            nc.sync.dma_start(mask_sb[:, m, :], cmask[m].bitcast(F32R))
        nc.sync.dma_start(ident_sb[:], ident[:, :])
        nc.vector.memset(ones_sb[:].bitcast(F32), 1.0)

        for b in range(B):
            qt = per_b.tile([64, G, S], F32R, tag="qt")
            kt = per_b.tile([64, S], F32R, tag="kt")
            vsb = per_b.tile([128, NJC, DH + 1], F32R, tag="vsb")
            at = per_b.tile([128, 2, S], F32R, tag="at")
            nc.vector.memset(vsb[:].bitcast(F32), 1.0)

            # ---------- fused QKV projection + rope + transposes ----------
            # Transposes for tile tt are emitted after tile tt+1's matmuls so
            # the PE never waits on the ACT-evict -> DVE-rope chain.
            def emit_tail(tt, qkv):
                tsl = slice(tt * 128, (tt + 1) * 128)
                for h in range(5):
                    ptr = pp_misc.tile([64, 128], F32, tag="misc")
                    nc.tensor.transpose(ptr[:], qkv[:, h * 64:(h + 1) * 64],
                                        ident_sb[:, :])
                    if h < G:
                        nc.vector.tensor_copy(qt[:, h, tsl], ptr[:])
                    else:
                        nc.vector.tensor_copy(kt[:, tsl], ptr[:])
                nc.vector.tensor_copy(vsb[:, tt, 0:DH], qkv[:, 320:384])

            prev = None
            for tg in range(4):             # groups of 512 tokens, 4 psum accs
                pq = [pp_proj.tile([128, 384], F32, tag=f"pq{s}",
                                   name=f"pq{s}_{b}_{tg}")
                      for s in range(4)]
                for k in range(KC):
                    xbig = xpool.tile([128, 512], F32R, tag="xt")
                    nc.sync.dma_start(
                        xbig[:],
                        xt[b, k * 128:(k + 1) * 128,
                           tg * 512:(tg + 1) * 512].bitcast(F32R))
                    for s in range(4):
                        nc.tensor.matmul(pq[s][:],
                                         xbig[:, s * 128:(s + 1) * 128],
                                         wall_sb[:, k, :],
                                         start=(k == 0), stop=(k == KC - 1))
                for s in range(4):
                    tt = tg * 4 + s
                    qkv = qkvpool.tile([128, 384], F32, tag="qkv")
                    nc.scalar.copy(qkv[:], pq[s][:])
                    # rope on q+k (cols 0:320), interleaved pairs in free dim
                    pear = qkv[:, 0:320].rearrange("p (h i two) -> p h i two",
                                                   two=2, i=32)
                    ev, od = pear[:, :, :, 0], pear[:, :, :, 1]
                    cs = cos_sb[:, tt, :].rearrange("p (h i) -> p h i", i=32)
                    sn = sin_sb[:, tt, :].rearrange("p (h i) -> p h i", i=32)
                    ec = rpool.tile([128, 5, 32], F32, tag="ec")
                    es = rpool.tile([128, 5, 32], F32, tag="es")
                    oc = rpool.tile([128, 5, 32], F32, tag="oc")
                    os_ = rpool.tile([128, 5, 32], F32, tag="os")
                    nc.vector.tensor_mul(ec[:], ev, cs)
                    nc.vector.tensor_mul(es[:], ev, sn)
                    nc.vector.tensor_mul(oc[:], od, cs)
                    nc.vector.tensor_mul(os_[:], od, sn)
                    nc.vector.tensor_sub(ev, ec[:], os_[:])
                    nc.vector.tensor_add(od, es[:], oc[:])
                    if prev is not None:
                        emit_tail(*prev)
                    prev = (tt, qkv)
            emit_tail(*prev)

            # ---------- attention ----------
            for g in range(G):
                cc, r0 = g // 2, (g % 2) * 64
                for it in range(NTT):
                    isl = slice(it * TT, (it + 1) * TT)
                    pav = pp_av.tile([65, TT], F32, tag="av")
                    njc = 4 * it + 4
                    pending = []  # attn.V pipelined two steps behind scores
                    for jc in range(njc):
                        psc = pp_att.tile([128, TT], F32, tag="sc")
                        nc.tensor.matmul(
                            psc[:], kt[:, jc * 128:(jc + 1) * 128],
                            qt[:, g, isl], start=True, stop=True)
                        esb = epool.tile([128, TT], F32R, tag="exp")
                        nc.scalar.activation(esb[:], psc[:], EXP, scale=0.125)
                        if jc >= 4 * it:  # diagonal block: causal mask
                            nc.vector.tensor_mul(esb[:], esb[:],
                                                 mask_sb[:, jc - 4 * it, :])
                        pending.append(((pav[:], vsb[:, jc, :], esb[:]),
                                        dict(start=(jc == 0),
                                             stop=(jc == njc - 1))))
                        if len(pending) > 2:
                            a = pending.pop(0)
                            nc.tensor.matmul(*a[0], **a[1])
                    for a in pending:
                        nc.tensor.matmul(*a[0], **a[1])
                    # normalize via ones-row sum: recip -> PE broadcast -> mul
                    rcp = spool.tile([1, TT], F32, tag="rcp")
                    nc.vector.reciprocal(rcp[:], pav[64:65, :])
                    avs = spool.tile([64, TT], F32, tag="avs")
                    nc.scalar.copy(avs[:], pav[0:64, :])
                    rcpr = spool.tile([1, TT], F32R, tag="rcpr")
                    nc.vector.tensor_copy(rcpr[:], rcp[:])
                    pbc = pp_misc.tile([64, TT], F32, tag="misc")
                    nc.tensor.matmul(pbc[:], ones_sb[:], rcpr[:],
                                     start=True, stop=True)
                    nc.vector.tensor_mul(at[r0:r0 + 64, cc, isl],
                                         avs[:], pbc[:])

            # ---------- o projection (partial over this core's channels) ----
            for tt in range(NJC):
                tsl = slice(tt * 128, (tt + 1) * 128)
                for nt in range(D // TT):
                    nsl = slice(nt * TT, (nt + 1) * TT)
                    po = pp_proj.tile([128, TT], F32, tag=f"pq{nt}",
                                      name=f"po{b}_{tt}_{nt}")
                    nc.tensor.matmul(po[:], at[:, 0, tsl], wot_sb[:, 0, nsl],
                                     start=True, stop=False)
                    nc.tensor.matmul(po[:], at[:, 1, tsl], wot_sb[:, 1, nsl],
                                     start=False, stop=True)
                    osb = opool.tile([128, TT], F32, tag="osb")
                    nc.vector.tensor_copy(osb[:], po[:])
                    nc.sync.dma_start(o[b, tsl, nsl], osb[:])
    nc.compile()
    return nc


def host_inputs(x, Wq, Wk, Wv, Wo):
    """Per-core input maps. Q/K weight rows permuted so each head is
    [interleaved] kept natural; rope works on interleaved pairs in the
    free dim, so NO permutation is needed here."""
    xtp = np.ascontiguousarray(np.transpose(np.asarray(x, np.float32), (0, 2, 1)))
    inv = ROPE_BASE ** (-np.arange(0, DH, 2, dtype=np.float64) / DH)
    th = np.arange(S, dtype=np.float64)[:, None] * inv[None, :]  # (S, 32)
    cosr = np.tile(np.cos(th), (1, 5)).astype(np.float32)  # (S, 160)
    sinr = np.tile(np.sin(th), (1, 5)).astype(np.float32)
    p = np.arange(128)[:, None]
    f = np.arange(TT)[None, :]
    cmask = np.stack([(p + m * 128 <= f).astype(np.float32) for m in range(4)])
    ident = np.eye(128, dtype=np.float32)
    in_maps = []
    for c in range(NCORES):
        wall = np.concatenate([Wq[256 * c:256 * (c + 1)],
                               Wk[DH * c:DH * (c + 1)],
                               Wv[DH * c:DH * (c + 1)]], axis=0)
        wall = np.ascontiguousarray(wall.T.astype(np.float32))       # (D, 384)
        wot = np.ascontiguousarray(Wo[:, 256 * c:256 * (c + 1)].T
                                   .astype(np.float32))              # (256, D)
        in_maps.append(dict(xt=xtp, wall=wall, wot=wot, cosr=cosr,
                            sinr=sinr, cmask=cmask, ident=ident))
    return in_maps


def kernel(**inputs):
    x = np.asarray(inputs["x"], dtype=np.float32)
    Wq = np.asarray(inputs["Wq"], dtype=np.float32)
    Wk = np.asarray(inputs["Wk"], dtype=np.float32)
    Wv = np.asarray(inputs["Wv"], dtype=np.float32)
    Wo = np.asarray(inputs["Wo"], dtype=np.float32)
    in_maps = host_inputs(x, Wq, Wk, Wv, Wo)
    if "nc" not in _cached:
        _cached["nc"] = build_nc()
    res = run_bass_kernel_spmd(_cached["nc"], in_maps, list(range(NCORES)))
    out = np.zeros((B, S, D), np.float64)
    for r in res.results:
        out += r["o"]
    return out.astype(np.float32)



# revision 3
# speedup vs baseline: 1.3590x; 1.3590x over previous
"""GQA (32q/8kv heads, RoPE, causal) TRN2 kernel v3.

Sharding: 8 cores = 2 batches x 4 kv-pairs. Core (b, kvp) owns batch b,
kv heads {2kvp, 2kvp+1}, q heads 8kvp..8kvp+7. Each core emits a partial
o [S, D] (bf16); host sums 4 partials per batch.

Precision: QKV projection in fp8e4 DoubleRow (0.5 cyc/row; contraction 2048
averages the quantization noise away). Attention core in bf16 (fp8 scores/
probs/V/at each cost 2-3% output error — the attention output is ~1/sqrt(n)
smaller than V so quantization noise does NOT average down relative to it).

Per-core pipeline:
  QKV proj (fp8 DR, token-layout) -> psum [128t, 1024]
  RoPE: 3 DVE ops (pair-swap with signed-sin table) -> qk bf16 [128, 640]
  PE transposes (bf16) -> qt [64, 8, S] / kt [64, 2, S]; V -> vsb (+ones col)
  Scores (bf16, trimmed to [qlo,512)): psc [128k, 512q]; diagonal gets a
    -240 triangular tile added via a second matmul into the same psum group
  exp on ACT (trimmed) -> probs bf16 [128, kb, 512]; Pool memsets [0,qlo)
  AV orientation-2: out = attn [128 tok, 65]: lhsT = probs block [128k, 128t]
    stationary, rhs = vsb [128k, 65] moving (65 free = full PE util); 4-head
    slabs [128, 4, 65] per psum bank; col 64 = denominator (per-partition!)
  Normalize: DVE recip [128,4] + one broadcast-free mul -> atq bf16 [128, 512]
  at transpose (PE) -> atT [128 chan, 4, S]; o-proj bf16 -> po [128, 512];
  Pool evicts to bf16, DMA out per token tile.
"""
import numpy as np
from contextlib import ExitStack

import concourse.bass as bass
from concourse import bacc
import concourse.mybir as mybir
import concourse.tile as tile
from concourse.bass_utils import run_bass_kernel_spmd
import ml_dtypes

F32 = mybir.dt.float32
BF16 = mybir.dt.bfloat16
FP8 = mybir.dt.float8e4
EXP = mybir.ActivationFunctionType.Exp
DR = mybir.MatmulPerfMode.DoubleRow

D = 2048
DH = 64
NCORES = 8
ROPE_BASE = 10000.0
MASKVAL = -240.0
EBIAS = -2.0     # probs = exp(0.125*scores - 2); cancels in normalization

_cached = {}


def build_nc(S=2048, dbg=False):
    NTT = S // 128
    NIT = S // 512
    KC = D // 128
    NH = 8
    nc = bacc.Bacc("TRN2", target_bir_lowering=False, debug=False)
    dbg_d = {}
    if dbg:
        dbg_d["d_qt"] = nc.declare_dram_parameter("d_qt", [64, NH, S], F32, isOutput=True)
        dbg_d["d_kt"] = nc.declare_dram_parameter("d_kt", [64, 2, S], F32, isOutput=True)
        dbg_d["d_vsb"] = nc.declare_dram_parameter("d_vsb", [128, 2, NTT, 65], F32, isOutput=True)
        dbg_d["d_at"] = nc.declare_dram_parameter("d_at", [128, 4, S], F32, isOutput=True)
        dbg_d["d_pb"] = nc.declare_dram_parameter("d_pb", [128, NTT, 512], F32, isOutput=True)
    xtb = nc.declare_dram_parameter("xtb", [NTT, 128, KC, 128], BF16, isOutput=False)
    wallb = nc.declare_dram_parameter("wallb", [128, KC, 768], BF16, isOutput=False)
    wotb = nc.declare_dram_parameter("wotb", [128, 4, D], BF16, isOutput=False)
    cosb = nc.declare_dram_parameter("cosb", [NTT, 128, 64], BF16, isOutput=False)
    sinsg = nc.declare_dram_parameter("sinsg", [NTT, 128, 64], BF16, isOutput=False)
    identf = nc.declare_dram_parameter("identf", [128, 128], F32, isOutput=False)
    trif = nc.declare_dram_parameter("trif", [128, 128], F32, isOutput=False)
    o = nc.declare_dram_parameter("o", [S, D], BF16, isOutput=True)

    with tile.TileContext(nc) as tc, ExitStack() as ctx:
        wp = ctx.enter_context(tc.tile_pool(name="weights", bufs=1))
        sp = ctx.enter_context(tc.tile_pool(name="state", bufs=1))
        xs = ctx.enter_context(tc.tile_pool(name="xstream", bufs=3))
        rp = ctx.enter_context(tc.tile_pool(name="ring", bufs=2))
        pr = ctx.enter_context(tc.tile_pool(name="probs", bufs=3))
        aq = ctx.enter_context(tc.tile_pool(name="atq", bufs=2))
        ob = ctx.enter_context(tc.tile_pool(name="osb", bufs=2))
        sm = ctx.enter_context(tc.tile_pool(name="small", bufs=4))

        # ---------- persistent weights / tables ----------
        wall = wp.tile([128, KC, 768], BF16, tag="wall")
        wot = wp.tile([128, 4, D], BF16, tag="wot")
        cos_sb = wp.tile([128, NTT, 64], BF16, tag="cos")
        sin_sb = wp.tile([128, NTT, 64], BF16, tag="sin")
        idb = wp.tile([128, 128], BF16, tag="idb")
        trib = wp.tile([128, 128], BF16, tag="trib")
        nbias = wp.tile([128, 1], F32, tag="nbias")

        nc.sync.dma_start(wall[:], wallb[:, :, :])
        nc.sync.dma_start(wot[:], wotb[:, :, :])
        nc.sync.dma_start(cos_sb[:], cosb[:, :, :].rearrange("tt p c -> p tt c"))
        nc.sync.dma_start(sin_sb[:], sinsg[:, :, :].rearrange("tt p c -> p tt c"))
        idf_s = sm.tile([128, 128], F32, tag="idf")
        trf_s = sm.tile([128, 128], F32, tag="trf")
        nc.sync.dma_start(idf_s[:], identf[:, :])
        nc.sync.dma_start(trf_s[:], trif[:, :])
        nc.vector.tensor_copy(idb[:], idf_s[:])
        nc.vector.tensor_copy(trib[:], trf_s[:])
        nc.vector.memset(nbias[:], EBIAS)

        # ---------- per-core state ----------
        qt = sp.tile([64, NH, S], BF16, tag="qt")
        kt = sp.tile([64, 2, S], BF16, tag="kt")
        vsb = sp.tile([128, 2, NTT, 65], BF16, tag="vsb")
        atT = sp.tile([128, 4, S], BF16, tag="atT")
        nc.vector.memset(vsb[:, :, :, 64:65], 1.0)

        # ================= phase 1: QKV + rope + transposes =================
        pq_pool = ExitStack()
        pp_qkv = pq_pool.enter_context(tc.tile_pool(name="pqkv", bufs=2, space="PSUM"))
        pp_tr = pq_pool.enter_context(tc.tile_pool(name="ptr", bufs=2, space="PSUM"))
        pre_pool = ExitStack()
        pp_pre = pre_pool.enter_context(tc.tile_pool(name="presc", bufs=1, space="PSUM"))

        pbs = [None] * NH
        prescored = set()

        def emit_scores(it, h, pool, scbufs):
            kv = h // 4
            nkb = 4 * it + 4
            i0 = it * 512
            tag = "probs0" if (it == 0 and h < 5 and NTT >= 16) else "probs"
            pbufs = 5 if tag == "probs0" else 3
            pb = pr.tile([128, nkb, 512], BF16, tag=tag, bufs=pbufs,
                         name=f"pb{it}_{h}")
            pbs[h] = pb
            for kb in range(nkb):
                diag = kb >= 4 * it
                qlo = (kb - 4 * it) * 128 if diag else 0
                psc = pool.tile([128, 512], F32, tag="sc", bufs=scbufs,
                                name=f"psc{it}_{h}_{kb}")
                nc.tensor.matmul(psc[:, qlo:512],
                                 kt[:, kv, kb * 128:(kb + 1) * 128],
                                 qt[:, h, i0 + qlo:i0 + 512],
                                 start=True, stop=not diag)
                if diag:
                    nc.tensor.matmul(psc[:, qlo:qlo + 128], idb[:], trib[:],
                                     start=False, stop=True)
                nc.scalar.activation(pb[:, kb, qlo:512], psc[:, qlo:512],
                                     EXP, scale=0.125, bias=nbias[:])
                if qlo:
                    nc.gpsimd.memset(pb[:, kb, 0:qlo], 0.0)

        def p1_tail(tt, qk8):
            tsl = slice(tt * 128, (tt + 1) * 128)
            qtr = pp_tr.tile([64, 8, 128], BF16, tag="qtr", name=f"qtr{tt}")
            ktr = pp_tr.tile([64, 8, 128], BF16, tag="qtr", name=f"ktr{tt}")
            ktr = ktr[:, 0:2, :]
            for h in range(8):
                nc.tensor.matmul(qtr[:, h, :], qk8[:, h * 64:(h + 1) * 64],
                                 idb[:], is_transpose=True,
                                 start=(h == 0), stop=(h == 7))
            for g in range(2):
                nc.tensor.matmul(ktr[:, g, :],
                                 qk8[:, 512 + g * 64:512 + (g + 1) * 64],
                                 idb[:], is_transpose=True,
                                 start=(g == 0), stop=(g == 1))
            nc.vector.tensor_copy(qt[:, :, tsl], qtr[:])
            nc.vector.tensor_copy(kt[:, :, tsl], ktr[:])

        prev = None
        for tt in range(NTT):
            xtile = xs.tile([128, KC, 128], BF16, tag="xt", name=f"xt{tt}")
            nc.sync.dma_start(xtile[:], xtb[tt])
            pq = pp_qkv.tile([128, 1024], F32, tag="pq", name=f"pq{tt}")
            for kp in range(KC):
                xv = xtile[:, kp, :]
                nc.tensor.matmul(pq[:, 0:512], xv, wall[:, kp, 0:512],
                                 start=(kp == 0), stop=(kp == KC - 1))
                nc.tensor.matmul(pq[:, 512:768], xv, wall[:, kp, 512:768],
                                 start=(kp == 0), stop=(kp == KC - 1))
            if prev is not None:
                p1_tail(*prev)
            if 5 <= tt < 10 and NTT >= 16:
                emit_scores(0, tt - 5, pp_pre, 2)
                prescored.add(tt - 5)
            # ACT evicts psum -> bf16 sbuf (GPSIMD cannot touch PSUM);
            # rope: op1 tmp = pairswap(qk)*sinsg (Pool); op2 t1 = qk*cos (Pool);
            # op3 qk8 = t1+tmp (DVE, bf16 2x)
            qkvb = rp.tile([128, 768], BF16, tag="qkvb")
            nc.scalar.copy(qkvb[:], pq[:, 0:768])
            qkv = qkvb[:, 0:640]
            swp = qkv.rearrange("p (h n two) -> p h n two", two=2, n=32)[..., ::-1]
            tmp = rp.tile([128, 640], BF16, tag="tmp")
            t1 = rp.tile([128, 640], BF16, tag="t1")
            qk8 = rp.tile([128, 640], BF16, tag="qk8")
            sin4 = sin_sb[:, tt, :].rearrange("p (one n two) -> p one n two",
                                              one=1, two=2).to_broadcast([128, 10, 32, 2])
            cos3 = cos_sb[:, tt, :].rearrange("p (one c) -> p one c",
                                              one=1).to_broadcast([128, 10, 64])
            nc.gpsimd.tensor_mul(tmp[:].rearrange("p (h n two) -> p h n two",
                                                  two=2, n=32),
                                 swp, sin4)
            nc.gpsimd.tensor_mul(t1[:].rearrange("p (h c) -> p h c", h=10),
                                 qkv.rearrange("p (h c) -> p h c", h=10), cos3)
            nc.vector.tensor_add(qk8[:], t1[:], tmp[:])
            nc.vector.tensor_copy(vsb[:, :, tt, 0:64],
                                  qkvb[:, 640:768].rearrange("p (kv c) -> p kv c",
                                                             kv=2))
            prev = (tt, qk8)
        p1_tail(*prev)
        pre_pool.close()
        pq_pool.close()

        # ================= phase 2+3: attention + o-proj =================
        pp_att = ctx.enter_context(tc.tile_pool(name="patt", bufs=1, space="PSUM"))
        avs = [None] * 2
        oproj_q = []

        for it in range(NIT):
            i0 = it * 512

            def emit_av(h):
                kv = h // 4
                pb = pbs[h]
                for tq in range(4):
                    tt = 4 * it + tq
                    if h % 2 == 0 and tq % 2 == 0:
                        avs[tq // 2] = pp_att.tile([128, 2, 2, 128], F32,
                                                   tag=f"avs{tq // 2}", bufs=1,
                                                   name=f"avs{it}_{h}_{tq}")
                    slab = avs[tq // 2]
                    for kb in range(tt + 1):
                        nc.tensor.matmul(slab[:, tq % 2, h % 2, 0:65],
                                         pb[:, kb, tq * 128:(tq + 1) * 128],
                                         vsb[:, kv, kb, :],
                                         start=(kb == 0), stop=(kb == tt),
                                         skip_group_check=True)

            def emit_norm(g):
                for tq in range(4):
                    tt = 4 * it + tq
                    tsl = slice(tt * 128, (tt + 1) * 128)
                    slab = avs[tq // 2][:, tq % 2]
                    rec = sm.tile([128, 2, 1], F32, tag="rec")
                    nc.vector.reciprocal(rec[:, :, 0], slab[:, :, 64])
                    atq = aq.tile([128, 2, 64], BF16, tag="atq",
                                  name=f"atq{it}_{g}_{tq}")
                    nc.vector.tensor_mul(atq[:], slab[:, :, 0:64],
                                         rec[:].to_broadcast([128, 2, 64]))
                    attr = pp_att.tile([128, 128], BF16, tag="sc", bufs=4,
                                       name=f"attr{it}_{g}_{tq}")
                    nc.tensor.matmul(attr[:],
                                     atq[:].rearrange("p f c -> p (f c)"),
                                     idb[:], is_transpose=True,
                                     start=True, stop=True)
                    nc.vector.tensor_copy(atT[:, g, tsl], attr[:])

            def drain_oproj(n):
                for _ in range(n):
                    if oproj_q:
                        oproj_q.pop(0)()

            def maybe_scores(h):
                if it == 0 and h in prescored:
                    return
                emit_scores(it, h, pp_att, 4)

            maybe_scores(0)
            drain_oproj(2)
            for h in range(1, NH):
                maybe_scores(h)
                emit_av(h - 1)
                if h % 2 == 0:
                    emit_norm(h // 2 - 1)
                drain_oproj(2)
            emit_av(NH - 1)
            emit_norm(NH // 2 - 1)

            def queue_oproj(it_):
                state = {}
                for tq in range(4):
                    tt = 4 * it_ + tq
                    for nt in range(4):
                        def step(tt=tt, nt=nt):
                            tsl = slice(tt * 128, (tt + 1) * 128)
                            if nt == 0:
                                state[tt] = ob.tile([128, D], BF16, tag="osb",
                                                    name=f"osb{tt}")
                            osb = state[tt]
                            nsl = slice(nt * 512, (nt + 1) * 512)
                            po = pp_att.tile([128, 512], F32, tag="po", bufs=2,
                                             name=f"po{tt}_{nt}")
                            for c in range(4):
                                nc.tensor.matmul(po[:], atT[:, c, tsl],
                                                 wot[:, c, nsl],
                                                 start=(c == 0), stop=(c == 3))
                            nc.vector.tensor_copy(osb[:, nsl], po[:])
                            if nt == 3:
                                nc.sync.dma_start(o[tsl, :], osb[:])
                        oproj_q.append(step)
            queue_oproj(it)
        while oproj_q:
            oproj_q.pop(0)()

        if dbg:
            dsc = ctx.enter_context(tc.tile_pool(name="dsc", bufs=1))
            for nm, t in [("d_qt", qt[:]), ("d_kt", kt[:]), ("d_vsb", vsb[:]),
                          ("d_at", atT[:]), ("d_pb", pbs[0][:])]:
                f = dsc.tile(list(t.shape), F32, tag="f" + nm, name="f" + nm)
                nc.vector.tensor_copy(f[:], t)
                nc.sync.dma_start(dbg_d[nm][tuple(slice(None) for _ in t.shape)], f[:])
    nc.compile()
    return nc


# ====================== host side ======================

def _fp8(x):
    return np.asarray(x, np.float32).astype(ml_dtypes.float8_e4m3)


def host_inputs(x, Wq, Wk, Wv, Wo, S=2048):
    NTT = S // 128
    KC = D // 128
    inv = ROPE_BASE ** (-np.arange(0, DH, 2, dtype=np.float64) / DH)
    th = np.arange(S, dtype=np.float64)[:, None] * inv[None, :]
    cos1 = np.repeat(np.cos(th), 2, axis=1)
    sin1 = np.sin(th)
    sinsg1 = np.empty((S, 64))
    sinsg1[:, 0::2] = -sin1
    sinsg1[:, 1::2] = sin1
    cosb = cos1.reshape(NTT, 128, 64).astype(ml_dtypes.bfloat16)
    sing = sinsg1.reshape(NTT, 128, 64).astype(ml_dtypes.bfloat16)
    identf = np.eye(128, dtype=np.float32)
    p = np.arange(128)[:, None]
    q = np.arange(128)[None, :]
    trif = np.where(p <= q, 0.0, MASKVAL).astype(np.float32)

    in_maps = []
    for c in range(NCORES):
        b, kvp = c // 4, c % 4
        xb = np.asarray(x[b], np.float32)
        xt = xb.astype(ml_dtypes.bfloat16).reshape(NTT, 128, KC, 128)
        xtb = np.ascontiguousarray(xt.transpose(0, 3, 2, 1))
        wq = Wq[512 * kvp:512 * (kvp + 1)]
        wk = Wk[128 * kvp:128 * (kvp + 1)]
        wv = Wv[128 * kvp:128 * (kvp + 1)]
        wall = np.concatenate([wq, wk, wv], axis=0)
        wallb = np.ascontiguousarray(
            wall.astype(ml_dtypes.bfloat16).T.reshape(KC, 128, 768)
            .transpose(1, 0, 2))
        wotb = np.ascontiguousarray(
            Wo[:, 512 * kvp:512 * (kvp + 1)].astype(ml_dtypes.bfloat16)
            .T.reshape(4, 128, D).transpose(1, 0, 2))
        in_maps.append(dict(xtb=xtb, wallb=wallb, wotb=wotb, cosb=cosb,
                            sinsg=sing, identf=identf, trif=trif))
    return in_maps


def kernel(**inputs):
    x = np.asarray(inputs["x"], dtype=np.float32)
    Wq = np.asarray(inputs["Wq"], dtype=np.float32)
    Wk = np.asarray(inputs["Wk"], dtype=np.float32)
    Wv = np.asarray(inputs["Wv"], dtype=np.float32)
    Wo = np.asarray(inputs["Wo"], dtype=np.float32)
    B, S, _ = x.shape
    in_maps = host_inputs(x, Wq, Wk, Wv, Wo, S=S)
    if "nc" not in _cached:
        _cached["nc"] = build_nc(S=S)
    res = run_bass_kernel_spmd(_cached["nc"], in_maps, list(range(NCORES)))
    out = np.zeros((B, S, D), np.float64)
    for c, r in enumerate(res.results):
        out[c // 4] += np.asarray(r["o"], np.float32)
    return out.astype(np.float32)


# revision 4
# speedup vs baseline: 1.4527x; 1.0690x over previous
"""GQA (32q/8kv heads, RoPE, causal) TRN2 kernel v3.

Sharding: 8 cores = 2 batches x 4 kv-pairs. Core (b, kvp) owns batch b,
kv heads {2kvp, 2kvp+1}, q heads 8kvp..8kvp+7. Each core emits a partial
o [S, D] (bf16); host sums 4 partials per batch.

Precision: QKV projection in fp8e4 DoubleRow (0.5 cyc/row; contraction 2048
averages the quantization noise away). Attention core in bf16 (fp8 scores/
probs/V/at each cost 2-3% output error — the attention output is ~1/sqrt(n)
smaller than V so quantization noise does NOT average down relative to it).

Per-core pipeline:
  QKV proj (fp8 DR, token-layout) -> psum [128t, 1024]
  RoPE: 3 DVE ops (pair-swap with signed-sin table) -> qk bf16 [128, 640]
  PE transposes (bf16) -> qt [64, 8, S] / kt [64, 2, S]; V -> vsb (+ones col)
  Scores (bf16, trimmed to [qlo,512)): psc [128k, 512q]; diagonal gets a
    -240 triangular tile added via a second matmul into the same psum group
  exp on ACT (trimmed) -> probs bf16 [128, kb, 512]; Pool memsets [0,qlo)
  AV orientation-2: out = attn [128 tok, 65]: lhsT = probs block [128k, 128t]
    stationary, rhs = vsb [128k, 65] moving (65 free = full PE util); 4-head
    slabs [128, 4, 65] per psum bank; col 64 = denominator (per-partition!)
  Normalize: DVE recip [128,4] + one broadcast-free mul -> atq bf16 [128, 512]
  at transpose (PE) -> atT [128 chan, 4, S]; o-proj bf16 -> po [128, 512];
  Pool evicts to bf16, DMA out per token tile.
"""
import numpy as np
from contextlib import ExitStack

import concourse.bass as bass
from concourse import bacc
import concourse.mybir as mybir
import concourse.tile as tile
from concourse.bass_utils import run_bass_kernel_spmd
import ml_dtypes

F32 = mybir.dt.float32
BF16 = mybir.dt.bfloat16
FP8 = mybir.dt.float8e4
EXP = mybir.ActivationFunctionType.Exp
DR = mybir.MatmulPerfMode.DoubleRow

D = 2048
DH = 64
NCORES = 8
ROPE_BASE = 10000.0
MASKVAL = -240.0
EBIAS = -2.0     # probs = exp(0.125*scores - 2); cancels in normalization

_cached = {}


def build_nc(S=2048, dbg=False):
    NTT = S // 128
    NIT = S // 512
    KC = D // 128
    NH = 8
    nc = bacc.Bacc("TRN2", target_bir_lowering=False, debug=False)
    dbg_d = {}
    if dbg:
        dbg_d["d_qt"] = nc.declare_dram_parameter("d_qt", [64, NH, S], F32, isOutput=True)
        dbg_d["d_kt"] = nc.declare_dram_parameter("d_kt", [64, 2, S], F32, isOutput=True)
        dbg_d["d_vsb"] = nc.declare_dram_parameter("d_vsb", [128, 2, NTT, 65], F32, isOutput=True)
        dbg_d["d_at"] = nc.declare_dram_parameter("d_at", [128, 4, S], F32, isOutput=True)
        dbg_d["d_pb"] = nc.declare_dram_parameter("d_pb", [128, NTT, 512], F32, isOutput=True)
    xt8 = nc.declare_dram_parameter("xt8", [NTT, 128, KC, 2, 128], FP8, isOutput=False)
    wall8 = nc.declare_dram_parameter("wall8", [128, KC, 2, 768], FP8, isOutput=False)
    wotb = nc.declare_dram_parameter("wotb", [128, 4, D], BF16, isOutput=False)
    cosb = nc.declare_dram_parameter("cosb", [NTT, 128, 64], BF16, isOutput=False)
    sinsg = nc.declare_dram_parameter("sinsg", [NTT, 128, 64], BF16, isOutput=False)
    identf = nc.declare_dram_parameter("identf", [128, 128], F32, isOutput=False)
    trif = nc.declare_dram_parameter("trif", [128, 128], F32, isOutput=False)
    o = nc.declare_dram_parameter("o", [S, D], BF16, isOutput=True)

    with tile.TileContext(nc) as tc, ExitStack() as ctx:
        wp = ctx.enter_context(tc.tile_pool(name="weights", bufs=1))
        sp = ctx.enter_context(tc.tile_pool(name="state", bufs=1))
        xs = ctx.enter_context(tc.tile_pool(name="xstream", bufs=3))
        rp = ctx.enter_context(tc.tile_pool(name="ring", bufs=2))
        pr = ctx.enter_context(tc.tile_pool(name="probs", bufs=3))
        aq = ctx.enter_context(tc.tile_pool(name="atq", bufs=2))
        ob = ctx.enter_context(tc.tile_pool(name="osb", bufs=2))
        sm = ctx.enter_context(tc.tile_pool(name="small", bufs=4))

        # ---------- persistent weights / tables ----------
        wall = wp.tile([128, KC, 2, 768], FP8, tag="wall")
        wot = wp.tile([128, 4, D], BF16, tag="wot")
        cos_sb = wp.tile([128, NTT, 64], BF16, tag="cos")
        sin_sb = wp.tile([128, NTT, 64], BF16, tag="sin")
        idb = wp.tile([128, 128], BF16, tag="idb")
        trib = wp.tile([128, 128], BF16, tag="trib")
        nbias = wp.tile([128, 1], F32, tag="nbias")

        nc.sync.dma_start(wall[:], wall8[:, :, :, :])
        nc.sync.dma_start(wot[:], wotb[:, :, :])
        nc.sync.dma_start(cos_sb[:], cosb[:, :, :].rearrange("tt p c -> p tt c"))
        nc.sync.dma_start(sin_sb[:], sinsg[:, :, :].rearrange("tt p c -> p tt c"))
        idf_s = sm.tile([128, 128], F32, tag="idf")
        trf_s = sm.tile([128, 128], F32, tag="trf")
        nc.sync.dma_start(idf_s[:], identf[:, :])
        nc.sync.dma_start(trf_s[:], trif[:, :])
        nc.vector.tensor_copy(idb[:], idf_s[:])
        nc.vector.tensor_copy(trib[:], trf_s[:])
        nc.vector.memset(nbias[:], EBIAS)

        # ---------- per-core state ----------
        qt = sp.tile([64, NH, S], BF16, tag="qt")
        kt = sp.tile([64, 2, S], BF16, tag="kt")
        vsb = sp.tile([128, 2, NTT, 65], BF16, tag="vsb")
        atT = sp.tile([128, 4, S], BF16, tag="atT")
        nc.vector.memset(vsb[:, :, :, 64:65], 1.0)

        # ================= phase 1: QKV + rope + transposes =================
        pq_pool = ExitStack()
        pp_qkv = pq_pool.enter_context(tc.tile_pool(name="pqkv", bufs=2, space="PSUM"))
        pp_tr = pq_pool.enter_context(tc.tile_pool(name="ptr", bufs=2, space="PSUM"))
        pre_pool = ExitStack()
        pp_pre = pre_pool.enter_context(tc.tile_pool(name="presc", bufs=1, space="PSUM"))

        pbs = [None] * NH
        prescored = set()

        def emit_scores(it, h, pool, scbufs):
            kv = h // 4
            nkb = 4 * it + 4
            i0 = it * 512
            tag = "probs0" if (it == 0 and h < 5 and NTT >= 16) else "probs"
            pbufs = 5 if tag == "probs0" else 3
            pb = pr.tile([128, nkb, 512], BF16, tag=tag, bufs=pbufs,
                         name=f"pb{it}_{h}")
            pbs[h] = pb
            for kb in range(nkb):
                diag = kb >= 4 * it
                qlo = (kb - 4 * it) * 128 if diag else 0
                psc = pool.tile([128, 512], F32, tag="sc", bufs=scbufs,
                                name=f"psc{it}_{h}_{kb}")
                nc.tensor.matmul(psc[:, qlo:512],
                                 kt[:, kv, kb * 128:(kb + 1) * 128],
                                 qt[:, h, i0 + qlo:i0 + 512],
                                 start=True, stop=not diag)
                if diag:
                    nc.tensor.matmul(psc[:, qlo:qlo + 128], idb[:], trib[:],
                                     start=False, stop=True)
                nc.scalar.activation(pb[:, kb, qlo:512], psc[:, qlo:512],
                                     EXP, scale=0.125, bias=nbias[:])
                if qlo:
                    nc.gpsimd.memset(pb[:, kb, 0:qlo], 0.0)

        def p1_tail(tt, qk8):
            tsl = slice(tt * 128, (tt + 1) * 128)
            qtr = pp_tr.tile([64, 8, 128], BF16, tag="qtr", name=f"qtr{tt}")
            ktr = pp_tr.tile([64, 8, 128], BF16, tag="qtr", name=f"ktr{tt}")
            ktr = ktr[:, 0:2, :]
            for h in range(8):
                nc.tensor.matmul(qtr[:, h, :], qk8[:, h * 64:(h + 1) * 64],
                                 idb[:], is_transpose=True,
                                 start=(h == 0), stop=(h == 7))
            for g in range(2):
                nc.tensor.matmul(ktr[:, g, :],
                                 qk8[:, 512 + g * 64:512 + (g + 1) * 64],
                                 idb[:], is_transpose=True,
                                 start=(g == 0), stop=(g == 1))
            nc.vector.tensor_copy(qt[:, :, tsl], qtr[:])
            nc.vector.tensor_copy(kt[:, :, tsl], ktr[:])

        prev = None
        for tt in range(NTT):
            xtile = xs.tile([128, KC, 2, 128], FP8, tag="xt", name=f"xt{tt}")
            nc.sync.dma_start(xtile[:], xt8[tt])
            pq = pp_qkv.tile([128, 1024], F32, tag="pq", name=f"pq{tt}")
            # exact-ish fp8 hi/lo split: (xh+xl)(wh+wl) ~ xh wh + xl wh + xh wl
            # per chunk-pair: 3 DoubleRow matmuls (0.75x bf16 cost); xl*wl dropped
            for kp in range(KC // 2):
                xh = xtile[:, 2 * kp:2 * kp + 2, 0, :]
                xl = xtile[:, 2 * kp:2 * kp + 2, 1, :]
                for c0, c1 in ((0, 512), (512, 768)):
                    wh = wall[:, 2 * kp:2 * kp + 2, 0, c0:c1]
                    wl = wall[:, 2 * kp:2 * kp + 2, 1, c0:c1]
                    st = (kp == 0)
                    nc.tensor.matmul(pq[:, c0:c1], xh, wh, start=st, stop=False,
                                     perf_mode=DR)
                    nc.tensor.matmul(pq[:, c0:c1], xl, wh, start=False, stop=False,
                                     perf_mode=DR)
                    nc.tensor.matmul(pq[:, c0:c1], xh, wl, start=False,
                                     stop=(kp == KC // 2 - 1), perf_mode=DR)
            if prev is not None:
                p1_tail(*prev)
            if 5 <= tt < 10 and NTT >= 16:
                emit_scores(0, tt - 5, pp_pre, 2)
                prescored.add(tt - 5)
            # ACT evicts psum -> bf16 sbuf (GPSIMD cannot touch PSUM);
            # rope: op1 tmp = pairswap(qk)*sinsg (Pool); op2 t1 = qk*cos (Pool);
            # op3 qk8 = t1+tmp (DVE, bf16 2x)
            qkvb = rp.tile([128, 768], BF16, tag="qkvb")
            nc.scalar.mul(qkvb[:], pq[:, 0:768], 1.0 / 64.0)
            qkv = qkvb[:, 0:640]
            swp = qkv.rearrange("p (h n two) -> p h n two", two=2, n=32)[..., ::-1]
            tmp = rp.tile([128, 640], BF16, tag="tmp")
            t1 = rp.tile([128, 640], BF16, tag="t1")
            qk8 = rp.tile([128, 640], BF16, tag="qk8")
            sin4 = sin_sb[:, tt, :].rearrange("p (one n two) -> p one n two",
                                              one=1, two=2).to_broadcast([128, 10, 32, 2])
            cos3 = cos_sb[:, tt, :].rearrange("p (one c) -> p one c",
                                              one=1).to_broadcast([128, 10, 64])
            nc.gpsimd.tensor_mul(tmp[:].rearrange("p (h n two) -> p h n two",
                                                  two=2, n=32),
                                 swp, sin4)
            nc.gpsimd.tensor_mul(t1[:].rearrange("p (h c) -> p h c", h=10),
                                 qkv.rearrange("p (h c) -> p h c", h=10), cos3)
            nc.vector.tensor_add(qk8[:], t1[:], tmp[:])
            nc.vector.tensor_copy(vsb[:, :, tt, 0:64],
                                  qkvb[:, 640:768].rearrange("p (kv c) -> p kv c",
                                                             kv=2))
            prev = (tt, qk8)
        p1_tail(*prev)
        pre_pool.close()
        pq_pool.close()

        # ================= phase 2+3: attention + o-proj =================
        pp_att = ctx.enter_context(tc.tile_pool(name="patt", bufs=1, space="PSUM"))
        avs = [None] * 2
        oproj_q = []

        for it in range(NIT):
            i0 = it * 512

            def emit_av(h):
                kv = h // 4
                pb = pbs[h]
                for tq in range(4):
                    tt = 4 * it + tq
                    if h % 2 == 0 and tq % 2 == 0:
                        avs[tq // 2] = pp_att.tile([128, 2, 2, 128], F32,
                                                   tag=f"avs{tq // 2}", bufs=1,
                                                   name=f"avs{it}_{h}_{tq}")
                    slab = avs[tq // 2]
                    for kb in range(tt + 1):
                        nc.tensor.matmul(slab[:, tq % 2, h % 2, 0:65],
                                         pb[:, kb, tq * 128:(tq + 1) * 128],
                                         vsb[:, kv, kb, :],
                                         start=(kb == 0), stop=(kb == tt),
                                         skip_group_check=True)

            def emit_norm(g):
                for tq in range(4):
                    tt = 4 * it + tq
                    tsl = slice(tt * 128, (tt + 1) * 128)
                    slab = avs[tq // 2][:, tq % 2]
                    rec = sm.tile([128, 2, 1], F32, tag="rec")
                    nc.vector.reciprocal(rec[:, :, 0], slab[:, :, 64])
                    atq = aq.tile([128, 2, 64], BF16, tag="atq",
                                  name=f"atq{it}_{g}_{tq}")
                    nc.vector.tensor_mul(atq[:], slab[:, :, 0:64],
                                         rec[:].to_broadcast([128, 2, 64]))
                    attr = pp_att.tile([128, 128], BF16, tag="sc", bufs=4,
                                       name=f"attr{it}_{g}_{tq}")
                    nc.tensor.matmul(attr[:],
                                     atq[:].rearrange("p f c -> p (f c)"),
                                     idb[:], is_transpose=True,
                                     start=True, stop=True)
                    nc.vector.tensor_copy(atT[:, g, tsl], attr[:])

            def drain_oproj(n):
                for _ in range(n):
                    if oproj_q:
                        oproj_q.pop(0)()

            def maybe_scores(h):
                if it == 0 and h in prescored:
                    return
                emit_scores(it, h, pp_att, 4)

            maybe_scores(0)
            drain_oproj(2)
            for h in range(1, NH):
                maybe_scores(h)
                emit_av(h - 1)
                if h % 2 == 0:
                    emit_norm(h // 2 - 1)
                drain_oproj(2)
            emit_av(NH - 1)
            emit_norm(NH // 2 - 1)

            def queue_oproj(it_):
                state = {}
                for tq in range(4):
                    tt = 4 * it_ + tq
                    for nt in range(4):
                        def step(tt=tt, nt=nt):
                            tsl = slice(tt * 128, (tt + 1) * 128)
                            if nt == 0:
                                state[tt] = ob.tile([128, D], BF16, tag="osb",
                                                    name=f"osb{tt}")
                            osb = state[tt]
                            nsl = slice(nt * 512, (nt + 1) * 512)
                            po = pp_att.tile([128, 512], F32, tag="po", bufs=2,
                                             name=f"po{tt}_{nt}")
                            for c in range(4):
                                nc.tensor.matmul(po[:], atT[:, c, tsl],
                                                 wot[:, c, nsl],
                                                 start=(c == 0), stop=(c == 3))
                            nc.vector.tensor_copy(osb[:, nsl], po[:])
                            if nt == 3:
                                nc.sync.dma_start(o[tsl, :], osb[:])
                        oproj_q.append(step)
            queue_oproj(it)
        while oproj_q:
            oproj_q.pop(0)()

        if dbg:
            dsc = ctx.enter_context(tc.tile_pool(name="dsc", bufs=1))
            for nm, t in [("d_qt", qt[:]), ("d_kt", kt[:]), ("d_vsb", vsb[:]),
                          ("d_at", atT[:]), ("d_pb", pbs[0][:])]:
                f = dsc.tile(list(t.shape), F32, tag="f" + nm, name="f" + nm)
                nc.vector.tensor_copy(f[:], t)
                nc.sync.dma_start(dbg_d[nm][tuple(slice(None) for _ in t.shape)], f[:])
    nc.compile()
    return nc


# ====================== host side ======================

def _fp8(x):
    return np.asarray(x, np.float32).astype(ml_dtypes.float8_e4m3)


def host_inputs(x, Wq, Wk, Wv, Wo, S=2048):
    NTT = S // 128
    KC = D // 128
    inv = ROPE_BASE ** (-np.arange(0, DH, 2, dtype=np.float64) / DH)
    th = np.arange(S, dtype=np.float64)[:, None] * inv[None, :]
    cos1 = np.repeat(np.cos(th), 2, axis=1)
    sin1 = np.sin(th)
    sinsg1 = np.empty((S, 64))
    sinsg1[:, 0::2] = -sin1
    sinsg1[:, 1::2] = sin1
    cosb = cos1.reshape(NTT, 128, 64).astype(ml_dtypes.bfloat16)
    sing = sinsg1.reshape(NTT, 128, 64).astype(ml_dtypes.bfloat16)
    identf = np.eye(128, dtype=np.float32)
    p = np.arange(128)[:, None]
    q = np.arange(128)[None, :]
    trif = np.where(p <= q, 0.0, MASKVAL).astype(np.float32)

    in_maps = []
    for c in range(NCORES):
        b, kvp = c // 4, c % 4
        xb = np.asarray(x[b], np.float32)
        xh = _fp8(xb)
        xl = _fp8(xb - xh.astype(np.float32))
        xt = np.stack([xh, xl], axis=0).reshape(2, NTT, 128, KC, 128)
        xt8 = np.ascontiguousarray(xt.transpose(1, 4, 3, 0, 2))
        wq = Wq[512 * kvp:512 * (kvp + 1)]
        wk = Wk[128 * kvp:128 * (kvp + 1)]
        wv = Wv[128 * kvp:128 * (kvp + 1)]
        wall = np.concatenate([wq, wk, wv], axis=0) * 64.0
        wh = _fp8(wall)
        wl = _fp8(wall - wh.astype(np.float32))
        wall8 = np.ascontiguousarray(
            np.stack([wh, wl], axis=0).transpose(2, 0, 1)
            .reshape(KC, 128, 2, 768).transpose(1, 0, 2, 3))
        wotb = np.ascontiguousarray(
            Wo[:, 512 * kvp:512 * (kvp + 1)].astype(ml_dtypes.bfloat16)
            .T.reshape(4, 128, D).transpose(1, 0, 2))
        in_maps.append(dict(xt8=xt8, wall8=wall8, wotb=wotb, cosb=cosb,
                            sinsg=sing, identf=identf, trif=trif))
    return in_maps


def kernel(**inputs):
    x = np.asarray(inputs["x"], dtype=np.float32)
    Wq = np.asarray(inputs["Wq"], dtype=np.float32)
    Wk = np.asarray(inputs["Wk"], dtype=np.float32)
    Wv = np.asarray(inputs["Wv"], dtype=np.float32)
    Wo = np.asarray(inputs["Wo"], dtype=np.float32)
    B, S, _ = x.shape
    in_maps = host_inputs(x, Wq, Wk, Wv, Wo, S=S)
    if "nc" not in _cached:
        _cached["nc"] = build_nc(S=S)
    res = run_bass_kernel_spmd(_cached["nc"], in_maps, list(range(NCORES)))
    out = np.zeros((B, S, D), np.float64)
    for c, r in enumerate(res.results):
        out[c // 4] += np.asarray(r["o"], np.float32)
    return out.astype(np.float32)


# revision 5
# speedup vs baseline: 1.4547x; 1.0014x over previous
"""GQA (32q/8kv heads, RoPE, causal) TRN2 kernel v3.

Sharding: 8 cores = 2 batches x 4 kv-pairs. Core (b, kvp) owns batch b,
kv heads {2kvp, 2kvp+1}, q heads 8kvp..8kvp+7. Each core emits a partial
o [S, D] (bf16); host sums 4 partials per batch.

Precision: QKV projection in fp8e4 DoubleRow (0.5 cyc/row; contraction 2048
averages the quantization noise away). Attention core in bf16 (fp8 scores/
probs/V/at each cost 2-3% output error — the attention output is ~1/sqrt(n)
smaller than V so quantization noise does NOT average down relative to it).

Per-core pipeline:
  QKV proj (fp8 DR, token-layout) -> psum [128t, 1024]
  RoPE: 3 DVE ops (pair-swap with signed-sin table) -> qk bf16 [128, 640]
  PE transposes (bf16) -> qt [64, 8, S] / kt [64, 2, S]; V -> vsb (+ones col)
  Scores (bf16, trimmed to [qlo,512)): psc [128k, 512q]; diagonal gets a
    -240 triangular tile added via a second matmul into the same psum group
  exp on ACT (trimmed) -> probs bf16 [128, kb, 512]; Pool memsets [0,qlo)
  AV orientation-2: out = attn [128 tok, 65]: lhsT = probs block [128k, 128t]
    stationary, rhs = vsb [128k, 65] moving (65 free = full PE util); 4-head
    slabs [128, 4, 65] per psum bank; col 64 = denominator (per-partition!)
  Normalize: DVE recip [128,4] + one broadcast-free mul -> atq bf16 [128, 512]
  at transpose (PE) -> atT [128 chan, 4, S]; o-proj bf16 -> po [128, 512];
  Pool evicts to bf16, DMA out per token tile.
"""
import numpy as np
from contextlib import ExitStack

import concourse.bass as bass
from concourse import bacc
import concourse.mybir as mybir
import concourse.tile as tile
from concourse.bass_utils import run_bass_kernel_spmd
import ml_dtypes

F32 = mybir.dt.float32
BF16 = mybir.dt.bfloat16
FP8 = mybir.dt.float8e4
EXP = mybir.ActivationFunctionType.Exp
DR = mybir.MatmulPerfMode.DoubleRow

D = 2048
DH = 64
NCORES = 8
ROPE_BASE = 10000.0
MASKVAL = -240.0
EBIAS = -2.0     # probs = exp(0.125*scores - 2); cancels in normalization

_cached = {}


def build_nc(S=2048, dbg=False):
    NTT = S // 128
    NIT = S // 512
    KC = D // 128
    NH = 8
    nc = bacc.Bacc("TRN2", target_bir_lowering=False, debug=False)
    dbg_d = {}
    if dbg:
        dbg_d["d_qt"] = nc.declare_dram_parameter("d_qt", [64, NH, S], F32, isOutput=True)
        dbg_d["d_kt"] = nc.declare_dram_parameter("d_kt", [64, 2, S], F32, isOutput=True)
        dbg_d["d_vsb"] = nc.declare_dram_parameter("d_vsb", [128, 2, NTT, 65], F32, isOutput=True)
        dbg_d["d_at"] = nc.declare_dram_parameter("d_at", [128, 4, S], F32, isOutput=True)
        dbg_d["d_pb"] = nc.declare_dram_parameter("d_pb", [128, NTT, 512], F32, isOutput=True)
    xt8 = nc.declare_dram_parameter("xt8", [NTT, 128, KC, 2, 128], FP8, isOutput=False)
    wall8 = nc.declare_dram_parameter("wall8", [128, KC, 2, 768], FP8, isOutput=False)
    wotb = nc.declare_dram_parameter("wotb", [128, 4, D], BF16, isOutput=False)
    cosb = nc.declare_dram_parameter("cosb", [NTT, 128, 64], BF16, isOutput=False)
    sinsg = nc.declare_dram_parameter("sinsg", [NTT, 128, 64], BF16, isOutput=False)
    identf = nc.declare_dram_parameter("identf", [128, 128], F32, isOutput=False)
    trif = nc.declare_dram_parameter("trif", [128, 128], F32, isOutput=False)
    o = nc.declare_dram_parameter("o", [S, D], BF16, isOutput=True)

    with tile.TileContext(nc) as tc, ExitStack() as ctx:
        wp = ctx.enter_context(tc.tile_pool(name="weights", bufs=1))
        sp = ctx.enter_context(tc.tile_pool(name="state", bufs=1))
        xs = ctx.enter_context(tc.tile_pool(name="xstream", bufs=3))
        rp = ctx.enter_context(tc.tile_pool(name="ring", bufs=2))
        pr = ctx.enter_context(tc.tile_pool(name="probs", bufs=3))
        aq = ctx.enter_context(tc.tile_pool(name="atq", bufs=2))
        ob = ctx.enter_context(tc.tile_pool(name="osb", bufs=2))
        sm = ctx.enter_context(tc.tile_pool(name="small", bufs=4))

        # ---------- persistent weights / tables ----------
        wall = wp.tile([128, KC, 2, 768], FP8, tag="wall")
        wot = wp.tile([128, 4, D], BF16, tag="wot")
        cos_sb = wp.tile([128, NTT, 64], BF16, tag="cos")
        sin_sb = wp.tile([128, NTT, 64], BF16, tag="sin")
        idb = wp.tile([128, 128], BF16, tag="idb")
        trib = wp.tile([128, 128], BF16, tag="trib")
        nbias = wp.tile([128, 1], F32, tag="nbias")

        nc.sync.dma_start(wall[:], wall8[:, :, :, :])
        nc.sync.dma_start(wot[:], wotb[:, :, :])
        nc.sync.dma_start(cos_sb[:], cosb[:, :, :].rearrange("tt p c -> p tt c"))
        nc.sync.dma_start(sin_sb[:], sinsg[:, :, :].rearrange("tt p c -> p tt c"))
        idf_s = sm.tile([128, 128], F32, tag="idf")
        trf_s = sm.tile([128, 128], F32, tag="trf")
        nc.sync.dma_start(idf_s[:], identf[:, :])
        nc.sync.dma_start(trf_s[:], trif[:, :])
        nc.vector.tensor_copy(idb[:], idf_s[:])
        nc.vector.tensor_copy(trib[:], trf_s[:])
        nc.vector.memset(nbias[:], EBIAS)

        # ---------- per-core state ----------
        qt = sp.tile([64, NH, S], BF16, tag="qt")
        kt = sp.tile([64, 2, S], BF16, tag="kt")
        vsb = sp.tile([128, 2, NTT, 65], BF16, tag="vsb")
        atT = sp.tile([128, 4, S], BF16, tag="atT")
        nc.vector.memset(vsb[:, :, :, 64:65], 1.0)

        # ================= phase 1: QKV + rope + transposes =================
        pq_pool = ExitStack()
        pp_qkv = pq_pool.enter_context(tc.tile_pool(name="pqkv", bufs=2, space="PSUM"))
        pp_tr = pq_pool.enter_context(tc.tile_pool(name="ptr", bufs=2, space="PSUM"))
        pre_pool = ExitStack()
        pp_pre = pre_pool.enter_context(tc.tile_pool(name="presc", bufs=1, space="PSUM"))

        pbs = [None] * NH
        prescored = set()

        def emit_scores(it, h, pool, scbufs):
            kv = h // 4
            nkb = 4 * it + 4
            i0 = it * 512
            tag = "probs0" if (it == 0 and h < 5 and NTT >= 16) else "probs"
            pbufs = 5 if tag == "probs0" else 3
            pb = pr.tile([128, nkb, 512], BF16, tag=tag, bufs=pbufs,
                         name=f"pb{it}_{h}")
            pbs[h] = pb
            for kb in range(nkb):
                diag = kb >= 4 * it
                qlo = (kb - 4 * it) * 128 if diag else 0
                psc = pool.tile([128, 512], F32, tag="sc", bufs=scbufs,
                                name=f"psc{it}_{h}_{kb}")
                nc.tensor.matmul(psc[:, qlo:512],
                                 kt[:, kv, kb * 128:(kb + 1) * 128],
                                 qt[:, h, i0 + qlo:i0 + 512],
                                 start=True, stop=not diag)
                if diag:
                    nc.tensor.matmul(psc[:, qlo:qlo + 128], idb[:], trib[:],
                                     start=False, stop=True)
                nc.scalar.activation(pb[:, kb, qlo:512], psc[:, qlo:512],
                                     EXP, scale=0.125, bias=nbias[:])
                if qlo:
                    nc.gpsimd.memset(pb[:, kb, 0:qlo], 0.0)

        def p1_tail(tt, qk8):
            tsl = slice(tt * 128, (tt + 1) * 128)
            qtr = pp_tr.tile([64, 8, 128], BF16, tag="qtr", name=f"qtr{tt}")
            ktr = pp_tr.tile([64, 8, 128], BF16, tag="qtr", name=f"ktr{tt}")
            ktr = ktr[:, 0:2, :]
            for h in range(8):
                nc.tensor.matmul(qtr[:, h, :], qk8[:, h * 64:(h + 1) * 64],
                                 idb[:], is_transpose=True,
                                 start=(h == 0), stop=(h == 7))
            for g in range(2):
                nc.tensor.matmul(ktr[:, g, :],
                                 qk8[:, 512 + g * 64:512 + (g + 1) * 64],
                                 idb[:], is_transpose=True,
                                 start=(g == 0), stop=(g == 1))
            nc.vector.tensor_copy(qt[:, :, tsl], qtr[:])
            nc.vector.tensor_copy(kt[:, :, tsl], ktr[:])

        prev = None
        for tt in range(NTT):
            xtile = xs.tile([128, KC, 2, 128], FP8, tag="xt", name=f"xt{tt}")
            nc.sync.dma_start(xtile[:], xt8[tt])
            pq = pp_qkv.tile([128, 1024], F32, tag="pq", name=f"pq{tt}")
            # exact-ish fp8 hi/lo split: (xh+xl)(wh+wl) ~ xh wh + xl wh + xh wl
            # per chunk-pair: 3 DoubleRow matmuls (0.75x bf16 cost); xl*wl dropped
            for kp in range(KC // 2):
                xh = xtile[:, 2 * kp:2 * kp + 2, 0, :]
                xl = xtile[:, 2 * kp:2 * kp + 2, 1, :]
                for c0, c1 in ((0, 512), (512, 768)):
                    wh = wall[:, 2 * kp:2 * kp + 2, 0, c0:c1]
                    wl = wall[:, 2 * kp:2 * kp + 2, 1, c0:c1]
                    st = (kp == 0)
                    nc.tensor.matmul(pq[:, c0:c1], xh, wh, start=st, stop=False,
                                     perf_mode=DR)
                    nc.tensor.matmul(pq[:, c0:c1], xl, wh, start=False, stop=False,
                                     perf_mode=DR)
                    nc.tensor.matmul(pq[:, c0:c1], xh, wl, start=False,
                                     stop=(kp == KC // 2 - 1), perf_mode=DR)
            if prev is not None:
                p1_tail(*prev)
            if 5 <= tt < 10 and NTT >= 16:
                emit_scores(0, tt - 5, pp_pre, 2)
                prescored.add(tt - 5)
            # ACT evicts psum -> bf16 sbuf (GPSIMD cannot touch PSUM);
            # rope: op1 tmp = pairswap(qk)*sinsg (Pool); op2 t1 = qk*cos (Pool);
            # op3 qk8 = t1+tmp (DVE, bf16 2x)
            qkvb = rp.tile([128, 768], BF16, tag="qkvb")
            nc.vector.tensor_scalar_mul(qkvb[:], pq[:, 0:768], 1.0 / 64.0)
            qkv = qkvb[:, 0:640]
            swp = qkv.rearrange("p (h n two) -> p h n two", two=2, n=32)[..., ::-1]
            tmp = rp.tile([128, 640], BF16, tag="tmp")
            t1 = rp.tile([128, 640], BF16, tag="t1")
            qk8 = rp.tile([128, 640], BF16, tag="qk8")
            sin4 = sin_sb[:, tt, :].rearrange("p (one n two) -> p one n two",
                                              one=1, two=2).to_broadcast([128, 10, 32, 2])
            cos3 = cos_sb[:, tt, :].rearrange("p (one c) -> p one c",
                                              one=1).to_broadcast([128, 10, 64])
            nc.gpsimd.tensor_mul(tmp[:].rearrange("p (h n two) -> p h n two",
                                                  two=2, n=32),
                                 swp, sin4)
            nc.gpsimd.tensor_mul(t1[:].rearrange("p (h c) -> p h c", h=10),
                                 qkv.rearrange("p (h c) -> p h c", h=10), cos3)
            nc.vector.tensor_add(qk8[:], t1[:], tmp[:])
            nc.vector.tensor_copy(vsb[:, :, tt, 0:64],
                                  qkvb[:, 640:768].rearrange("p (kv c) -> p kv c",
                                                             kv=2))
            prev = (tt, qk8)
        p1_tail(*prev)
        pre_pool.close()
        pq_pool.close()

        # ================= phase 2+3: attention + o-proj =================
        pp_att = ctx.enter_context(tc.tile_pool(name="patt", bufs=1, space="PSUM"))
        avs = [None] * 2
        oproj_q = []

        for it in range(NIT):
            i0 = it * 512

            def emit_av(h):
                kv = h // 4
                pb = pbs[h]
                for tq in range(4):
                    tt = 4 * it + tq
                    if h % 2 == 0 and tq % 2 == 0:
                        avs[tq // 2] = pp_att.tile([128, 2, 2, 128], F32,
                                                   tag=f"avs{tq // 2}", bufs=1,
                                                   name=f"avs{it}_{h}_{tq}")
                    slab = avs[tq // 2]
                    for kb in range(tt + 1):
                        nc.tensor.matmul(slab[:, tq % 2, h % 2, 0:65],
                                         pb[:, kb, tq * 128:(tq + 1) * 128],
                                         vsb[:, kv, kb, :],
                                         start=(kb == 0), stop=(kb == tt),
                                         skip_group_check=True)

            def emit_norm(g):
                for tq in range(4):
                    tt = 4 * it + tq
                    tsl = slice(tt * 128, (tt + 1) * 128)
                    slab = avs[tq // 2][:, tq % 2]
                    rec = sm.tile([128, 2, 1], F32, tag="rec")
                    nc.vector.reciprocal(rec[:, :, 0], slab[:, :, 64])
                    atq = aq.tile([128, 2, 64], BF16, tag="atq",
                                  name=f"atq{it}_{g}_{tq}")
                    nc.vector.tensor_mul(atq[:], slab[:, :, 0:64],
                                         rec[:].to_broadcast([128, 2, 64]))
                    attr = pp_att.tile([128, 128], BF16, tag="sc", bufs=4,
                                       name=f"attr{it}_{g}_{tq}")
                    nc.tensor.matmul(attr[:],
                                     atq[:].rearrange("p f c -> p (f c)"),
                                     idb[:], is_transpose=True,
                                     start=True, stop=True)
                    nc.vector.tensor_copy(atT[:, g, tsl], attr[:])

            def drain_oproj(n):
                for _ in range(n):
                    if oproj_q:
                        oproj_q.pop(0)()

            def maybe_scores(h):
                if it == 0 and h in prescored:
                    return
                emit_scores(it, h, pp_att, 4)

            maybe_scores(0)
            drain_oproj(2)
            for h in range(1, NH):
                maybe_scores(h)
                emit_av(h - 1)
                if h % 2 == 0:
                    emit_norm(h // 2 - 1)
                drain_oproj(2)
            emit_av(NH - 1)
            emit_norm(NH // 2 - 1)

            def queue_oproj(it_):
                state = {}
                for tq in range(4):
                    tt = 4 * it_ + tq
                    for nt in range(4):
                        def step(tt=tt, nt=nt):
                            tsl = slice(tt * 128, (tt + 1) * 128)
                            if nt == 0:
                                state[tt] = ob.tile([128, D], BF16, tag="osb",
                                                    name=f"osb{tt}")
                            osb = state[tt]
                            nsl = slice(nt * 512, (nt + 1) * 512)
                            po = pp_att.tile([128, 512], F32, tag="po", bufs=2,
                                             name=f"po{tt}_{nt}")
                            for c in range(4):
                                nc.tensor.matmul(po[:], atT[:, c, tsl],
                                                 wot[:, c, nsl],
                                                 start=(c == 0), stop=(c == 3))
                            nc.vector.tensor_copy(osb[:, nsl], po[:])
                            if nt == 3:
                                nc.sync.dma_start(o[tsl, :], osb[:])
                        oproj_q.append(step)
            queue_oproj(it)
        while oproj_q:
            oproj_q.pop(0)()

        if dbg:
            dsc = ctx.enter_context(tc.tile_pool(name="dsc", bufs=1))
            for nm, t in [("d_qt", qt[:]), ("d_kt", kt[:]), ("d_vsb", vsb[:]),
                          ("d_at", atT[:]), ("d_pb", pbs[0][:])]:
                f = dsc.tile(list(t.shape), F32, tag="f" + nm, name="f" + nm)
                nc.vector.tensor_copy(f[:], t)
                nc.sync.dma_start(dbg_d[nm][tuple(slice(None) for _ in t.shape)], f[:])
    nc.compile()
    return nc


# ====================== host side ======================

def _fp8(x):
    return np.asarray(x, np.float32).astype(ml_dtypes.float8_e4m3)


def host_inputs(x, Wq, Wk, Wv, Wo, S=2048):
    NTT = S // 128
    KC = D // 128
    inv = ROPE_BASE ** (-np.arange(0, DH, 2, dtype=np.float64) / DH)
    th = np.arange(S, dtype=np.float64)[:, None] * inv[None, :]
    cos1 = np.repeat(np.cos(th), 2, axis=1)
    sin1 = np.sin(th)
    sinsg1 = np.empty((S, 64))
    sinsg1[:, 0::2] = -sin1
    sinsg1[:, 1::2] = sin1
    cosb = cos1.reshape(NTT, 128, 64).astype(ml_dtypes.bfloat16)
    sing = sinsg1.reshape(NTT, 128, 64).astype(ml_dtypes.bfloat16)
    identf = np.eye(128, dtype=np.float32)
    p = np.arange(128)[:, None]
    q = np.arange(128)[None, :]
    trif = np.where(p <= q, 0.0, MASKVAL).astype(np.float32)

    in_maps = []
    for c in range(NCORES):
        b, kvp = c // 4, c % 4
        xb = np.asarray(x[b], np.float32)
        xh = _fp8(xb)
        xl = _fp8(xb - xh.astype(np.float32))
        xt = np.stack([xh, xl], axis=0).reshape(2, NTT, 128, KC, 128)
        xt8 = np.ascontiguousarray(xt.transpose(1, 4, 3, 0, 2))
        wq = Wq[512 * kvp:512 * (kvp + 1)]
        wk = Wk[128 * kvp:128 * (kvp + 1)]
        wv = Wv[128 * kvp:128 * (kvp + 1)]
        wall = np.concatenate([wq, wk, wv], axis=0) * 64.0
        wh = _fp8(wall)
        wl = _fp8(wall - wh.astype(np.float32))
        wall8 = np.ascontiguousarray(
            np.stack([wh, wl], axis=0).transpose(2, 0, 1)
            .reshape(KC, 128, 2, 768).transpose(1, 0, 2, 3))
        wotb = np.ascontiguousarray(
            Wo[:, 512 * kvp:512 * (kvp + 1)].astype(ml_dtypes.bfloat16)
            .T.reshape(4, 128, D).transpose(1, 0, 2))
        in_maps.append(dict(xt8=xt8, wall8=wall8, wotb=wotb, cosb=cosb,
                            sinsg=sing, identf=identf, trif=trif))
    return in_maps


def kernel(**inputs):
    x = np.asarray(inputs["x"], dtype=np.float32)
    Wq = np.asarray(inputs["Wq"], dtype=np.float32)
    Wk = np.asarray(inputs["Wk"], dtype=np.float32)
    Wv = np.asarray(inputs["Wv"], dtype=np.float32)
    Wo = np.asarray(inputs["Wo"], dtype=np.float32)
    B, S, _ = x.shape
    in_maps = host_inputs(x, Wq, Wk, Wv, Wo, S=S)
    if "nc" not in _cached:
        _cached["nc"] = build_nc(S=S)
    res = run_bass_kernel_spmd(_cached["nc"], in_maps, list(range(NCORES)))
    out = np.zeros((B, S, D), np.float64)
    for c, r in enumerate(res.results):
        out[c // 4] += np.asarray(r["o"], np.float32)
    return out.astype(np.float32)


# revision 6
# speedup vs baseline: 1.4648x; 1.0069x over previous
"""GQA (32q/8kv heads, RoPE, causal) TRN2 kernel v3.

Sharding: 8 cores = 2 batches x 4 kv-pairs. Core (b, kvp) owns batch b,
kv heads {2kvp, 2kvp+1}, q heads 8kvp..8kvp+7. Each core emits a partial
o [S, D] (bf16); host sums 4 partials per batch.

Precision: QKV projection in fp8e4 DoubleRow (0.5 cyc/row; contraction 2048
averages the quantization noise away). Attention core in bf16 (fp8 scores/
probs/V/at each cost 2-3% output error — the attention output is ~1/sqrt(n)
smaller than V so quantization noise does NOT average down relative to it).

Per-core pipeline:
  QKV proj (fp8 DR, token-layout) -> psum [128t, 1024]
  RoPE: 3 DVE ops (pair-swap with signed-sin table) -> qk bf16 [128, 640]
  PE transposes (bf16) -> qt [64, 8, S] / kt [64, 2, S]; V -> vsb (+ones col)
  Scores (bf16, trimmed to [qlo,512)): psc [128k, 512q]; diagonal gets a
    -240 triangular tile added via a second matmul into the same psum group
  exp on ACT (trimmed) -> probs bf16 [128, kb, 512]; Pool memsets [0,qlo)
  AV orientation-2: out = attn [128 tok, 65]: lhsT = probs block [128k, 128t]
    stationary, rhs = vsb [128k, 65] moving (65 free = full PE util); 4-head
    slabs [128, 4, 65] per psum bank; col 64 = denominator (per-partition!)
  Normalize: DVE recip [128,4] + one broadcast-free mul -> atq bf16 [128, 512]
  at transpose (PE) -> atT [128 chan, 4, S]; o-proj bf16 -> po [128, 512];
  Pool evicts to bf16, DMA out per token tile.
"""
import numpy as np
from contextlib import ExitStack

import concourse.bass as bass
from concourse import bacc
import concourse.mybir as mybir
import concourse.tile as tile
from concourse.bass_utils import run_bass_kernel_spmd
import ml_dtypes

F32 = mybir.dt.float32
BF16 = mybir.dt.bfloat16
FP8 = mybir.dt.float8e4
EXP = mybir.ActivationFunctionType.Exp
DR = mybir.MatmulPerfMode.DoubleRow

D = 2048
DH = 64
NCORES = 8
ROPE_BASE = 10000.0
MASKVAL = -240.0
EBIAS = -2.0     # probs = exp(0.125*scores - 2); cancels in normalization

_cached = {}


def build_nc(S=2048, dbg=False):
    NTT = S // 128
    NIT = S // 512
    KC = D // 128
    NH = 8
    nc = bacc.Bacc("TRN2", target_bir_lowering=False, debug=False)
    dbg_d = {}
    if dbg:
        dbg_d["d_qt"] = nc.declare_dram_parameter("d_qt", [64, NH, S], F32, isOutput=True)
        dbg_d["d_kt"] = nc.declare_dram_parameter("d_kt", [64, 2, S], F32, isOutput=True)
        dbg_d["d_vsb"] = nc.declare_dram_parameter("d_vsb", [128, 2, NTT, 65], F32, isOutput=True)
        dbg_d["d_at"] = nc.declare_dram_parameter("d_at", [128, 4, S], F32, isOutput=True)
        dbg_d["d_pb"] = nc.declare_dram_parameter("d_pb", [128, NTT, 512], F32, isOutput=True)
    xt8 = nc.declare_dram_parameter("xt8", [NTT, 128, KC, 2, 128], FP8, isOutput=False)
    wall8 = nc.declare_dram_parameter("wall8", [128, KC, 2, 768], FP8, isOutput=False)
    wotb = nc.declare_dram_parameter("wotb", [128, 4, D], BF16, isOutput=False)
    cosb = nc.declare_dram_parameter("cosb", [NTT, 128, 64], BF16, isOutput=False)
    sinsg = nc.declare_dram_parameter("sinsg", [NTT, 128, 64], BF16, isOutput=False)
    identf = nc.declare_dram_parameter("identf", [128, 128], F32, isOutput=False)
    trif = nc.declare_dram_parameter("trif", [128, 128], F32, isOutput=False)
    o = nc.declare_dram_parameter("o", [S, D], BF16, isOutput=True)

    with tile.TileContext(nc) as tc, ExitStack() as ctx:
        wp = ctx.enter_context(tc.tile_pool(name="weights", bufs=1))
        sp = ctx.enter_context(tc.tile_pool(name="state", bufs=1))
        xs = ctx.enter_context(tc.tile_pool(name="xstream", bufs=2))
        rp = ctx.enter_context(tc.tile_pool(name="ring", bufs=3))
        pr = ctx.enter_context(tc.tile_pool(name="probs", bufs=3))
        aq = ctx.enter_context(tc.tile_pool(name="atq", bufs=2))
        ob = ctx.enter_context(tc.tile_pool(name="osb", bufs=2))
        sm = ctx.enter_context(tc.tile_pool(name="small", bufs=2))

        # ---------- persistent weights / tables ----------
        wall = wp.tile([128, KC, 2, 768], FP8, tag="wall")
        wot = wp.tile([128, 4, D], BF16, tag="wot")
        cos_sb = wp.tile([128, NTT, 64], BF16, tag="cos")
        sin_sb = wp.tile([128, NTT, 64], BF16, tag="sin")
        idb = wp.tile([128, 128], BF16, tag="idb")
        trib = wp.tile([128, 128], BF16, tag="trib")
        nbias = wp.tile([128, 1], F32, tag="nbias")

        nc.sync.dma_start(wall[:], wall8[:, :, :, :])
        nc.sync.dma_start(wot[:], wotb[:, :, :])
        nc.sync.dma_start(cos_sb[:], cosb[:, :, :].rearrange("tt p c -> p tt c"))
        nc.sync.dma_start(sin_sb[:], sinsg[:, :, :].rearrange("tt p c -> p tt c"))
        idf_s = sm.tile([128, 128], F32, tag="idf")
        trf_s = sm.tile([128, 128], F32, tag="trf")
        nc.sync.dma_start(idf_s[:], identf[:, :])
        nc.sync.dma_start(trf_s[:], trif[:, :])
        nc.vector.tensor_copy(idb[:], idf_s[:])
        nc.vector.tensor_copy(trib[:], trf_s[:])
        nc.vector.memset(nbias[:], EBIAS)

        # ---------- per-core state ----------
        qt = sp.tile([64, NH, S], BF16, tag="qt")
        kt = sp.tile([64, 2, S], BF16, tag="kt")
        vsb = sp.tile([128, 2, NTT, 65], BF16, tag="vsb")
        atT = sp.tile([128, 4, S], BF16, tag="atT")
        nc.vector.memset(vsb[:, :, :, 64:65], 1.0)

        # ================= phase 1: QKV + rope + transposes =================
        pq_pool = ExitStack()
        pp_qkv = pq_pool.enter_context(tc.tile_pool(name="pqkv", bufs=2, space="PSUM"))
        pp_tr = pq_pool.enter_context(tc.tile_pool(name="ptr", bufs=2, space="PSUM"))
        pre_pool = ExitStack()
        pp_pre = pre_pool.enter_context(tc.tile_pool(name="presc", bufs=1, space="PSUM"))

        pbs = [None] * NH
        prescored = set()

        def emit_scores(it, h, pool, scbufs, drain=None):
            kv = h // 4
            nkb = 4 * it + 4
            i0 = it * 512
            tag = "probs0" if (it == 0 and h < 5 and NTT >= 16) else "probs"
            pbufs = 5 if tag == "probs0" else 3
            pb = pr.tile([128, nkb, 512], BF16, tag=tag, bufs=pbufs,
                         name=f"pb{it}_{h}")
            pbs[h] = pb
            for kb in range(nkb):
                if drain and kb and kb % (max(2, nkb // 2)) == 0:
                    drain(1)
                diag = kb >= 4 * it
                qlo = (kb - 4 * it) * 128 if diag else 0
                psc = pool.tile([128, 512], F32, tag="sc", bufs=scbufs,
                                name=f"psc{it}_{h}_{kb}")
                nc.tensor.matmul(psc[:, qlo:512],
                                 kt[:, kv, kb * 128:(kb + 1) * 128],
                                 qt[:, h, i0 + qlo:i0 + 512],
                                 start=True, stop=not diag)
                if diag:
                    nc.tensor.matmul(psc[:, qlo:qlo + 128], idb[:], trib[:],
                                     start=False, stop=True)
                nc.scalar.activation(pb[:, kb, qlo:512], psc[:, qlo:512],
                                     EXP, scale=0.125, bias=nbias[:])
                if qlo:
                    nc.gpsimd.memset(pb[:, kb, 0:qlo], 0.0)

        def p1_tail(tt, qk8):
            tsl = slice(tt * 128, (tt + 1) * 128)
            qtr = pp_tr.tile([64, 8, 128], BF16, tag="qtr", name=f"qtr{tt}")
            ktr = pp_tr.tile([64, 8, 128], BF16, tag="qtr", name=f"ktr{tt}")
            ktr = ktr[:, 0:2, :]
            for h in range(8):
                nc.tensor.matmul(qtr[:, h, :], qk8[:, h * 64:(h + 1) * 64],
                                 idb[:], is_transpose=True,
                                 start=(h == 0), stop=(h == 7))
            for g in range(2):
                nc.tensor.matmul(ktr[:, g, :],
                                 qk8[:, 512 + g * 64:512 + (g + 1) * 64],
                                 idb[:], is_transpose=True,
                                 start=(g == 0), stop=(g == 1))
            nc.vector.tensor_copy(qt[:, :, tsl], qtr[:])
            nc.vector.tensor_copy(kt[:, :, tsl], ktr[:])

        prevq = []
        for tt in range(NTT):
            xtile = xs.tile([128, KC, 2, 128], FP8, tag="xt", name=f"xt{tt}")
            nc.sync.dma_start(xtile[:], xt8[tt])
            pq = pp_qkv.tile([128, 1024], F32, tag="pq", name=f"pq{tt}")
            # exact-ish fp8 hi/lo split: (xh+xl)(wh+wl) ~ xh wh + xl wh + xh wl
            # per chunk-pair: 3 DoubleRow matmuls (0.75x bf16 cost); xl*wl dropped
            for kp in range(KC // 2):
                xh = xtile[:, 2 * kp:2 * kp + 2, 0, :]
                xl = xtile[:, 2 * kp:2 * kp + 2, 1, :]
                for c0, c1 in ((0, 512), (512, 768)):
                    wh = wall[:, 2 * kp:2 * kp + 2, 0, c0:c1]
                    wl = wall[:, 2 * kp:2 * kp + 2, 1, c0:c1]
                    st = (kp == 0)
                    nc.tensor.matmul(pq[:, c0:c1], xh, wh, start=st, stop=False,
                                     perf_mode=DR)
                    nc.tensor.matmul(pq[:, c0:c1], xl, wh, start=False, stop=False,
                                     perf_mode=DR)
                    nc.tensor.matmul(pq[:, c0:c1], xh, wl, start=False,
                                     stop=(kp == KC // 2 - 1), perf_mode=DR)
            if len(prevq) >= 2:
                p1_tail(*prevq.pop(0))
            if 5 <= tt < 10 and NTT >= 16:
                emit_scores(0, tt - 5, pp_pre, 2)
                prescored.add(tt - 5)
            # ACT evicts psum -> bf16 sbuf (GPSIMD cannot touch PSUM);
            # rope: op1 tmp = pairswap(qk)*sinsg (Pool); op2 t1 = qk*cos (Pool);
            # op3 qk8 = t1+tmp (DVE, bf16 2x)
            qkvb = rp.tile([128, 768], BF16, tag="qkvb")
            nc.vector.tensor_scalar_mul(qkvb[:], pq[:, 0:768], 1.0 / 64.0)
            qkv = qkvb[:, 0:640]
            swp = qkv.rearrange("p (h n two) -> p h n two", two=2, n=32)[..., ::-1]
            tmp = rp.tile([128, 640], BF16, tag="tmp")
            t1 = rp.tile([128, 640], BF16, tag="t1")
            qk8 = rp.tile([128, 640], BF16, tag="qk8")
            sin4 = sin_sb[:, tt, :].rearrange("p (one n two) -> p one n two",
                                              one=1, two=2).to_broadcast([128, 10, 32, 2])
            cos3 = cos_sb[:, tt, :].rearrange("p (one c) -> p one c",
                                              one=1).to_broadcast([128, 10, 64])
            nc.gpsimd.tensor_mul(tmp[:].rearrange("p (h n two) -> p h n two",
                                                  two=2, n=32),
                                 swp, sin4)
            nc.gpsimd.tensor_mul(t1[:].rearrange("p (h c) -> p h c", h=10),
                                 qkv.rearrange("p (h c) -> p h c", h=10), cos3)
            nc.vector.tensor_add(qk8[:], t1[:], tmp[:])
            nc.vector.tensor_copy(vsb[:, :, tt, 0:64],
                                  qkvb[:, 640:768].rearrange("p (kv c) -> p kv c",
                                                             kv=2))
            prevq.append((tt, qk8))
        for pv in prevq:
            p1_tail(*pv)
        pre_pool.close()
        pq_pool.close()

        # ================= phase 2+3: attention + o-proj =================
        pp_att = ctx.enter_context(tc.tile_pool(name="patt", bufs=1, space="PSUM"))
        avs = [None] * 2
        oproj_q = []

        for it in range(NIT):
            i0 = it * 512

            def emit_av(h):
                kv = h // 4
                pb = pbs[h]
                for tq in range(4):
                    tt = 4 * it + tq
                    if h % 2 == 0 and tq % 2 == 0:
                        avs[tq // 2] = pp_att.tile([128, 2, 2, 128], F32,
                                                   tag=f"avs{tq // 2}", bufs=1,
                                                   name=f"avs{it}_{h}_{tq}")
                    slab = avs[tq // 2]
                    for kb in range(tt + 1):
                        nc.tensor.matmul(slab[:, tq % 2, h % 2, 0:65],
                                         pb[:, kb, tq * 128:(tq + 1) * 128],
                                         vsb[:, kv, kb, :],
                                         start=(kb == 0), stop=(kb == tt),
                                         skip_group_check=True)

            def emit_norm(g):
                for tq in range(4):
                    tt = 4 * it + tq
                    tsl = slice(tt * 128, (tt + 1) * 128)
                    slab = avs[tq // 2][:, tq % 2]
                    rec = sm.tile([128, 2, 1], F32, tag="rec")
                    nc.vector.reciprocal(rec[:, :, 0], slab[:, :, 64])
                    atq = aq.tile([128, 2, 64], BF16, tag="atq",
                                  name=f"atq{it}_{g}_{tq}")
                    nc.vector.tensor_mul(atq[:], slab[:, :, 0:64],
                                         rec[:].to_broadcast([128, 2, 64]))
                    attr = pp_att.tile([128, 128], BF16, tag="sc", bufs=4,
                                       name=f"attr{it}_{g}_{tq}")
                    nc.tensor.matmul(attr[:],
                                     atq[:].rearrange("p f c -> p (f c)"),
                                     idb[:], is_transpose=True,
                                     start=True, stop=True)
                    nc.vector.tensor_copy(atT[:, g, tsl], attr[:])

            def drain_oproj(n):
                for _ in range(n):
                    if oproj_q:
                        oproj_q.pop(0)()

            def maybe_scores(h):
                if it == 0 and h in prescored:
                    return
                emit_scores(it, h, pp_att, 4, drain=drain_oproj)

            maybe_scores(0)
            drain_oproj(2)
            for h in range(1, NH):
                maybe_scores(h)
                emit_av(h - 1)
                if h % 2 == 0:
                    emit_norm(h // 2 - 1)
                drain_oproj(2)
            emit_av(NH - 1)
            emit_norm(NH // 2 - 1)

            def queue_oproj(it_):
                state = {}
                for tq in range(4):
                    tt = 4 * it_ + tq
                    for nt in range(4):
                        def step(tt=tt, nt=nt):
                            tsl = slice(tt * 128, (tt + 1) * 128)
                            if nt == 0:
                                state[tt] = ob.tile([128, D], BF16, tag="osb",
                                                    name=f"osb{tt}")
                            osb = state[tt]
                            nsl = slice(nt * 512, (nt + 1) * 512)
                            po = pp_att.tile([128, 512], F32, tag="po", bufs=2,
                                             name=f"po{tt}_{nt}")
                            for c in range(4):
                                nc.tensor.matmul(po[:], atT[:, c, tsl],
                                                 wot[:, c, nsl],
                                                 start=(c == 0), stop=(c == 3))
                            nc.vector.tensor_copy(osb[:, nsl], po[:])
                            if nt == 3:
                                nc.sync.dma_start(o[tsl, :], osb[:])
                        oproj_q.append(step)
            queue_oproj(it)
        while oproj_q:
            oproj_q.pop(0)()

        if dbg:
            dsc = ctx.enter_context(tc.tile_pool(name="dsc", bufs=1))
            for nm, t in [("d_qt", qt[:]), ("d_kt", kt[:]), ("d_vsb", vsb[:]),
                          ("d_at", atT[:]), ("d_pb", pbs[0][:])]:
                f = dsc.tile(list(t.shape), F32, tag="f" + nm, name="f" + nm)
                nc.vector.tensor_copy(f[:], t)
                nc.sync.dma_start(dbg_d[nm][tuple(slice(None) for _ in t.shape)], f[:])
    nc.compile()
    return nc


# ====================== host side ======================

def _fp8(x):
    return np.asarray(x, np.float32).astype(ml_dtypes.float8_e4m3)


def host_inputs(x, Wq, Wk, Wv, Wo, S=2048):
    NTT = S // 128
    KC = D // 128
    inv = ROPE_BASE ** (-np.arange(0, DH, 2, dtype=np.float64) / DH)
    th = np.arange(S, dtype=np.float64)[:, None] * inv[None, :]
    cos1 = np.repeat(np.cos(th), 2, axis=1)
    sin1 = np.sin(th)
    sinsg1 = np.empty((S, 64))
    sinsg1[:, 0::2] = -sin1
    sinsg1[:, 1::2] = sin1
    cosb = cos1.reshape(NTT, 128, 64).astype(ml_dtypes.bfloat16)
    sing = sinsg1.reshape(NTT, 128, 64).astype(ml_dtypes.bfloat16)
    identf = np.eye(128, dtype=np.float32)
    p = np.arange(128)[:, None]
    q = np.arange(128)[None, :]
    trif = np.where(p <= q, 0.0, MASKVAL).astype(np.float32)

    in_maps = []
    for c in range(NCORES):
        b, kvp = c // 4, c % 4
        xb = np.asarray(x[b], np.float32)
        xh = _fp8(xb)
        xl = _fp8(xb - xh.astype(np.float32))
        xt = np.stack([xh, xl], axis=0).reshape(2, NTT, 128, KC, 128)
        xt8 = np.ascontiguousarray(xt.transpose(1, 4, 3, 0, 2))
        wq = Wq[512 * kvp:512 * (kvp + 1)]
        wk = Wk[128 * kvp:128 * (kvp + 1)]
        wv = Wv[128 * kvp:128 * (kvp + 1)]
        wall = np.concatenate([wq, wk, wv], axis=0) * 64.0
        wh = _fp8(wall)
        wl = _fp8(wall - wh.astype(np.float32))
        wall8 = np.ascontiguousarray(
            np.stack([wh, wl], axis=0).transpose(2, 0, 1)
            .reshape(KC, 128, 2, 768).transpose(1, 0, 2, 3))
        wotb = np.ascontiguousarray(
            Wo[:, 512 * kvp:512 * (kvp + 1)].astype(ml_dtypes.bfloat16)
            .T.reshape(4, 128, D).transpose(1, 0, 2))
        in_maps.append(dict(xt8=xt8, wall8=wall8, wotb=wotb, cosb=cosb,
                            sinsg=sing, identf=identf, trif=trif))
    return in_maps


def kernel(**inputs):
    x = np.asarray(inputs["x"], dtype=np.float32)
    Wq = np.asarray(inputs["Wq"], dtype=np.float32)
    Wk = np.asarray(inputs["Wk"], dtype=np.float32)
    Wv = np.asarray(inputs["Wv"], dtype=np.float32)
    Wo = np.asarray(inputs["Wo"], dtype=np.float32)
    B, S, _ = x.shape
    in_maps = host_inputs(x, Wq, Wk, Wv, Wo, S=S)
    if "nc" not in _cached:
        _cached["nc"] = build_nc(S=S)
    res = run_bass_kernel_spmd(_cached["nc"], in_maps, list(range(NCORES)))
    out = np.zeros((B, S, D), np.float64)
    for c, r in enumerate(res.results):
        out[c // 4] += np.asarray(r["o"], np.float32)
    return out.astype(np.float32)


# revision 7
# speedup vs baseline: 1.4721x; 1.0050x over previous
"""GQA (32q/8kv heads, RoPE, causal) TRN2 kernel v3.

Sharding: 8 cores = 2 batches x 4 kv-pairs. Core (b, kvp) owns batch b,
kv heads {2kvp, 2kvp+1}, q heads 8kvp..8kvp+7. Each core emits a partial
o [S, D] (bf16); host sums 4 partials per batch.

Precision: QKV projection in fp8e4 DoubleRow (0.5 cyc/row; contraction 2048
averages the quantization noise away). Attention core in bf16 (fp8 scores/
probs/V/at each cost 2-3% output error — the attention output is ~1/sqrt(n)
smaller than V so quantization noise does NOT average down relative to it).

Per-core pipeline:
  QKV proj (fp8 DR, token-layout) -> psum [128t, 1024]
  RoPE: 3 DVE ops (pair-swap with signed-sin table) -> qk bf16 [128, 640]
  PE transposes (bf16) -> qt [64, 8, S] / kt [64, 2, S]; V -> vsb (+ones col)
  Scores (bf16, trimmed to [qlo,512)): psc [128k, 512q]; diagonal gets a
    -240 triangular tile added via a second matmul into the same psum group
  exp on ACT (trimmed) -> probs bf16 [128, kb, 512]; Pool memsets [0,qlo)
  AV orientation-2: out = attn [128 tok, 65]: lhsT = probs block [128k, 128t]
    stationary, rhs = vsb [128k, 65] moving (65 free = full PE util); 4-head
    slabs [128, 4, 65] per psum bank; col 64 = denominator (per-partition!)
  Normalize: DVE recip [128,4] + one broadcast-free mul -> atq bf16 [128, 512]
  at transpose (PE) -> atT [128 chan, 4, S]; o-proj bf16 -> po [128, 512];
  Pool evicts to bf16, DMA out per token tile.
"""
import numpy as np
from contextlib import ExitStack

import concourse.bass as bass
from concourse import bacc
import concourse.mybir as mybir
import concourse.tile as tile
from concourse.bass_utils import run_bass_kernel_spmd
import ml_dtypes

F32 = mybir.dt.float32
BF16 = mybir.dt.bfloat16
FP8 = mybir.dt.float8e4
EXP = mybir.ActivationFunctionType.Exp
DR = mybir.MatmulPerfMode.DoubleRow

D = 2048
DH = 64
NCORES = 8
ROPE_BASE = 10000.0
MASKVAL = -240.0
EBIAS = -2.0     # probs = exp(0.125*scores - 2); cancels in normalization

_cached = {}


def build_nc(S=2048, dbg=False):
    NTT = S // 128
    NIT = S // 512
    KC = D // 128
    NH = 8
    nc = bacc.Bacc("TRN2", target_bir_lowering=False, debug=False)
    dbg_d = {}
    if dbg:
        dbg_d["d_qt"] = nc.declare_dram_parameter("d_qt", [64, NH, S], F32, isOutput=True)
        dbg_d["d_kt"] = nc.declare_dram_parameter("d_kt", [64, 2, S], F32, isOutput=True)
        dbg_d["d_vsb"] = nc.declare_dram_parameter("d_vsb", [128, 2, NTT, 65], F32, isOutput=True)
        dbg_d["d_at"] = nc.declare_dram_parameter("d_at", [128, 4, S], F32, isOutput=True)
        dbg_d["d_pb"] = nc.declare_dram_parameter("d_pb", [128, NTT, 512], F32, isOutput=True)
    xt8 = nc.declare_dram_parameter("xt8", [NTT, 128, KC, 2, 128], FP8, isOutput=False)
    wall8 = nc.declare_dram_parameter("wall8", [128, KC, 2, 768], FP8, isOutput=False)
    wotb = nc.declare_dram_parameter("wotb", [128, 4, D], BF16, isOutput=False)
    cosb = nc.declare_dram_parameter("cosb", [NTT, 128, 64], BF16, isOutput=False)
    sinsg = nc.declare_dram_parameter("sinsg", [NTT, 128, 64], BF16, isOutput=False)
    identf = nc.declare_dram_parameter("identf", [128, 128], F32, isOutput=False)
    trif = nc.declare_dram_parameter("trif", [128, 128], F32, isOutput=False)
    o = nc.declare_dram_parameter("o", [S, D], BF16, isOutput=True)

    with tile.TileContext(nc) as tc, ExitStack() as ctx:
        wp = ctx.enter_context(tc.tile_pool(name="weights", bufs=1))
        sp = ctx.enter_context(tc.tile_pool(name="state", bufs=1))
        xs = ctx.enter_context(tc.tile_pool(name="xstream", bufs=2))
        rp = ctx.enter_context(tc.tile_pool(name="ring", bufs=3))
        pr = ctx.enter_context(tc.tile_pool(name="probs", bufs=3))
        aq = ctx.enter_context(tc.tile_pool(name="atq", bufs=2))
        ob = ctx.enter_context(tc.tile_pool(name="osb", bufs=2))
        sm = ctx.enter_context(tc.tile_pool(name="small", bufs=2))

        # ---------- persistent weights / tables ----------
        wall = wp.tile([128, KC, 2, 768], FP8, tag="wall")
        wot = wp.tile([128, 4, D], BF16, tag="wot")
        cos_sb = wp.tile([128, NTT, 64], BF16, tag="cos")
        sin_sb = wp.tile([128, NTT, 64], BF16, tag="sin")
        idb = wp.tile([128, 128], BF16, tag="idb")
        trib = wp.tile([128, 128], BF16, tag="trib")
        nbias = wp.tile([128, 1], F32, tag="nbias")

        nc.sync.dma_start(wall[:], wall8[:, :, :, :])
        nc.sync.dma_start(wot[:], wotb[:, :, :])
        nc.sync.dma_start(cos_sb[:], cosb[:, :, :].rearrange("tt p c -> p tt c"))
        nc.sync.dma_start(sin_sb[:], sinsg[:, :, :].rearrange("tt p c -> p tt c"))
        idf_s = sm.tile([128, 128], F32, tag="idf")
        trf_s = sm.tile([128, 128], F32, tag="trf")
        nc.sync.dma_start(idf_s[:], identf[:, :])
        nc.sync.dma_start(trf_s[:], trif[:, :])
        nc.vector.tensor_copy(idb[:], idf_s[:])
        nc.vector.tensor_copy(trib[:], trf_s[:])
        nc.vector.memset(nbias[:], EBIAS)

        # ---------- per-core state ----------
        qt = sp.tile([64, NH, S], BF16, tag="qt")
        kt = sp.tile([64, 2, S], BF16, tag="kt")
        vsb = sp.tile([128, 2, NTT, 65], BF16, tag="vsb")
        atT = sp.tile([128, 4, S], BF16, tag="atT")
        nc.vector.memset(vsb[:, :, :, 64:65], 1.0)

        # ================= phase 1: QKV + rope + transposes =================
        pq_pool = ExitStack()
        pp_qkv = pq_pool.enter_context(tc.tile_pool(name="pqkv", bufs=2, space="PSUM"))
        pp_tr = pq_pool.enter_context(tc.tile_pool(name="ptr", bufs=2, space="PSUM"))
        pre_pool = ExitStack()
        pp_pre = pre_pool.enter_context(tc.tile_pool(name="presc", bufs=1, space="PSUM"))

        pbs = [None] * NH
        prescored = set()

        def emit_scores(it, h, pool, scbufs, drain=None):
            kv = h // 4
            nkb = 4 * it + 4
            i0 = it * 512
            tag = "probs0" if (it == 0 and NTT >= 16) else "probs"
            pbufs = 8 if tag == "probs0" else 2
            pb = pr.tile([128, nkb, 512], BF16, tag=tag, bufs=pbufs,
                         name=f"pb{it}_{h}")
            pbs[h] = pb
            for kb in range(nkb):
                if drain and kb and kb % (max(2, nkb // 2)) == 0:
                    drain(1)
                diag = kb >= 4 * it
                qlo = (kb - 4 * it) * 128 if diag else 0
                psc = pool.tile([128, 512], F32, tag="sc", bufs=scbufs,
                                name=f"psc{it}_{h}_{kb}")
                nc.tensor.matmul(psc[:, qlo:512],
                                 kt[:, kv, kb * 128:(kb + 1) * 128],
                                 qt[:, h, i0 + qlo:i0 + 512],
                                 start=True, stop=not diag)
                if diag:
                    nc.tensor.matmul(psc[:, qlo:qlo + 128], idb[:], trib[:],
                                     start=False, stop=True)
                nc.scalar.activation(pb[:, kb, qlo:512], psc[:, qlo:512],
                                     EXP, scale=0.125, bias=nbias[:])
                if qlo:
                    nc.gpsimd.memset(pb[:, kb, 0:qlo], 0.0)

        def p1_tail(tt, qk8):
            tsl = slice(tt * 128, (tt + 1) * 128)
            qtr = pp_tr.tile([64, 8, 128], BF16, tag="qtr", name=f"qtr{tt}")
            ktr = pp_tr.tile([64, 8, 128], BF16, tag="qtr", name=f"ktr{tt}")
            ktr = ktr[:, 0:2, :]
            for h in range(8):
                nc.tensor.matmul(qtr[:, h, :], qk8[:, h * 64:(h + 1) * 64],
                                 idb[:], is_transpose=True,
                                 start=(h == 0), stop=(h == 7))
            for g in range(2):
                nc.tensor.matmul(ktr[:, g, :],
                                 qk8[:, 512 + g * 64:512 + (g + 1) * 64],
                                 idb[:], is_transpose=True,
                                 start=(g == 0), stop=(g == 1))
            nc.vector.tensor_copy(qt[:, :, tsl], qtr[:])
            nc.vector.tensor_copy(kt[:, :, tsl], ktr[:])

        prevq = []
        for tt in range(NTT):
            xtile = xs.tile([128, KC, 2, 128], FP8, tag="xt", name=f"xt{tt}")
            nc.sync.dma_start(xtile[:], xt8[tt])
            pq = pp_qkv.tile([128, 1024], F32, tag="pq", name=f"pq{tt}")
            # exact-ish fp8 hi/lo split: (xh+xl)(wh+wl) ~ xh wh + xl wh + xh wl
            # per chunk-pair: 3 DoubleRow matmuls (0.75x bf16 cost); xl*wl dropped
            for kp in range(KC // 2):
                xh = xtile[:, 2 * kp:2 * kp + 2, 0, :]
                xl = xtile[:, 2 * kp:2 * kp + 2, 1, :]
                for c0, c1 in ((0, 512), (512, 768)):
                    wh = wall[:, 2 * kp:2 * kp + 2, 0, c0:c1]
                    wl = wall[:, 2 * kp:2 * kp + 2, 1, c0:c1]
                    st = (kp == 0)
                    nc.tensor.matmul(pq[:, c0:c1], xh, wh, start=st, stop=False,
                                     perf_mode=DR)
                    nc.tensor.matmul(pq[:, c0:c1], xl, wh, start=False, stop=False,
                                     perf_mode=DR)
                    nc.tensor.matmul(pq[:, c0:c1], xh, wl, start=False,
                                     stop=(kp == KC // 2 - 1), perf_mode=DR)
            if len(prevq) >= 2:
                p1_tail(*prevq.pop(0))
            if 5 <= tt < 13 and NTT >= 16:
                emit_scores(0, tt - 5, pp_pre, 2)
                prescored.add(tt - 5)
            # ACT evicts psum -> bf16 sbuf (GPSIMD cannot touch PSUM);
            # rope: op1 tmp = pairswap(qk)*sinsg (Pool); op2 t1 = qk*cos (Pool);
            # op3 qk8 = t1+tmp (DVE, bf16 2x)
            qkvb = rp.tile([128, 768], BF16, tag="qkvb")
            nc.vector.tensor_scalar_mul(qkvb[:], pq[:, 0:768], 1.0 / 64.0)
            qkv = qkvb[:, 0:640]
            swp = qkv.rearrange("p (h n two) -> p h n two", two=2, n=32)[..., ::-1]
            tmp = rp.tile([128, 640], BF16, tag="tmp")
            t1 = rp.tile([128, 640], BF16, tag="t1")
            qk8 = rp.tile([128, 640], BF16, tag="qk8")
            sin4 = sin_sb[:, tt, :].rearrange("p (one n two) -> p one n two",
                                              one=1, two=2).to_broadcast([128, 10, 32, 2])
            cos3 = cos_sb[:, tt, :].rearrange("p (one c) -> p one c",
                                              one=1).to_broadcast([128, 10, 64])
            nc.gpsimd.tensor_mul(tmp[:].rearrange("p (h n two) -> p h n two",
                                                  two=2, n=32),
                                 swp, sin4)
            nc.gpsimd.tensor_mul(t1[:].rearrange("p (h c) -> p h c", h=10),
                                 qkv.rearrange("p (h c) -> p h c", h=10), cos3)
            nc.vector.tensor_add(qk8[:], t1[:], tmp[:])
            nc.vector.tensor_copy(vsb[:, :, tt, 0:64],
                                  qkvb[:, 640:768].rearrange("p (kv c) -> p kv c",
                                                             kv=2))
            prevq.append((tt, qk8))
        for pv in prevq:
            p1_tail(*pv)
        pre_pool.close()
        pq_pool.close()

        # ================= phase 2+3: attention + o-proj =================
        pp_att = ctx.enter_context(tc.tile_pool(name="patt", bufs=1, space="PSUM"))
        avs = [None] * 2
        oproj_q = []

        for it in range(NIT):
            i0 = it * 512

            def emit_av(h):
                kv = h // 4
                pb = pbs[h]
                for tq in range(4):
                    tt = 4 * it + tq
                    if h % 2 == 0 and tq % 2 == 0:
                        avs[tq // 2] = pp_att.tile([128, 2, 2, 128], F32,
                                                   tag=f"avs{tq // 2}", bufs=1,
                                                   name=f"avs{it}_{h}_{tq}")
                    slab = avs[tq // 2]
                    for kb in range(tt + 1):
                        nc.tensor.matmul(slab[:, tq % 2, h % 2, 0:65],
                                         pb[:, kb, tq * 128:(tq + 1) * 128],
                                         vsb[:, kv, kb, :],
                                         start=(kb == 0), stop=(kb == tt),
                                         skip_group_check=True)

            def emit_norm(g):
                for tq in range(4):
                    tt = 4 * it + tq
                    tsl = slice(tt * 128, (tt + 1) * 128)
                    slab = avs[tq // 2][:, tq % 2]
                    rec = sm.tile([128, 2, 1], F32, tag="rec")
                    nc.vector.reciprocal(rec[:, :, 0], slab[:, :, 64])
                    atq = aq.tile([128, 2, 64], BF16, tag="atq",
                                  name=f"atq{it}_{g}_{tq}")
                    nc.vector.tensor_mul(atq[:], slab[:, :, 0:64],
                                         rec[:].to_broadcast([128, 2, 64]))
                    attr = pp_att.tile([128, 128], BF16, tag="sc", bufs=4,
                                       name=f"attr{it}_{g}_{tq}")
                    nc.tensor.matmul(attr[:],
                                     atq[:].rearrange("p f c -> p (f c)"),
                                     idb[:], is_transpose=True,
                                     start=True, stop=True)
                    nc.vector.tensor_copy(atT[:, g, tsl], attr[:])

            def drain_oproj(n):
                for _ in range(n):
                    if oproj_q:
                        oproj_q.pop(0)()

            def maybe_scores(h):
                if it == 0 and h in prescored:
                    return
                emit_scores(it, h, pp_att, 4, drain=drain_oproj)

            maybe_scores(0)
            drain_oproj(2)
            for h in range(1, NH):
                maybe_scores(h)
                emit_av(h - 1)
                if h % 2 == 0:
                    emit_norm(h // 2 - 1)
                drain_oproj(2)
            emit_av(NH - 1)
            emit_norm(NH // 2 - 1)

            def queue_oproj(it_):
                state = {}
                for tq in range(4):
                    tt = 4 * it_ + tq
                    for nt in range(4):
                        def step(tt=tt, nt=nt):
                            tsl = slice(tt * 128, (tt + 1) * 128)
                            if nt == 0:
                                state[tt] = ob.tile([128, D], BF16, tag="osb",
                                                    name=f"osb{tt}")
                            osb = state[tt]
                            nsl = slice(nt * 512, (nt + 1) * 512)
                            po = pp_att.tile([128, 512], F32, tag="po", bufs=2,
                                             name=f"po{tt}_{nt}")
                            for c in range(4):
                                nc.tensor.matmul(po[:], atT[:, c, tsl],
                                                 wot[:, c, nsl],
                                                 start=(c == 0), stop=(c == 3))
                            nc.vector.tensor_copy(osb[:, nsl], po[:])
                            if nt == 3:
                                nc.sync.dma_start(o[tsl, :], osb[:])
                        oproj_q.append(step)
            queue_oproj(it)
        while oproj_q:
            oproj_q.pop(0)()

        if dbg:
            dsc = ctx.enter_context(tc.tile_pool(name="dsc", bufs=1))
            for nm, t in [("d_qt", qt[:]), ("d_kt", kt[:]), ("d_vsb", vsb[:]),
                          ("d_at", atT[:]), ("d_pb", pbs[0][:])]:
                f = dsc.tile(list(t.shape), F32, tag="f" + nm, name="f" + nm)
                nc.vector.tensor_copy(f[:], t)
                nc.sync.dma_start(dbg_d[nm][tuple(slice(None) for _ in t.shape)], f[:])
    nc.compile()
    return nc


# ====================== host side ======================

def _fp8(x):
    return np.asarray(x, np.float32).astype(ml_dtypes.float8_e4m3)


def host_inputs(x, Wq, Wk, Wv, Wo, S=2048):
    NTT = S // 128
    KC = D // 128
    inv = ROPE_BASE ** (-np.arange(0, DH, 2, dtype=np.float64) / DH)
    th = np.arange(S, dtype=np.float64)[:, None] * inv[None, :]
    cos1 = np.repeat(np.cos(th), 2, axis=1)
    sin1 = np.sin(th)
    sinsg1 = np.empty((S, 64))
    sinsg1[:, 0::2] = -sin1
    sinsg1[:, 1::2] = sin1
    cosb = cos1.reshape(NTT, 128, 64).astype(ml_dtypes.bfloat16)
    sing = sinsg1.reshape(NTT, 128, 64).astype(ml_dtypes.bfloat16)
    identf = np.eye(128, dtype=np.float32)
    p = np.arange(128)[:, None]
    q = np.arange(128)[None, :]
    trif = np.where(p <= q, 0.0, MASKVAL).astype(np.float32)

    in_maps = []
    for c in range(NCORES):
        b, kvp = c // 4, c % 4
        xb = np.asarray(x[b], np.float32)
        xh = _fp8(xb)
        xl = _fp8(xb - xh.astype(np.float32))
        xt = np.stack([xh, xl], axis=0).reshape(2, NTT, 128, KC, 128)
        xt8 = np.ascontiguousarray(xt.transpose(1, 4, 3, 0, 2))
        wq = Wq[512 * kvp:512 * (kvp + 1)]
        wk = Wk[128 * kvp:128 * (kvp + 1)]
        wv = Wv[128 * kvp:128 * (kvp + 1)]
        wall = np.concatenate([wq, wk, wv], axis=0) * 64.0
        wh = _fp8(wall)
        wl = _fp8(wall - wh.astype(np.float32))
        wall8 = np.ascontiguousarray(
            np.stack([wh, wl], axis=0).transpose(2, 0, 1)
            .reshape(KC, 128, 2, 768).transpose(1, 0, 2, 3))
        wotb = np.ascontiguousarray(
            Wo[:, 512 * kvp:512 * (kvp + 1)].astype(ml_dtypes.bfloat16)
            .T.reshape(4, 128, D).transpose(1, 0, 2))
        in_maps.append(dict(xt8=xt8, wall8=wall8, wotb=wotb, cosb=cosb,
                            sinsg=sing, identf=identf, trif=trif))
    return in_maps


def kernel(**inputs):
    x = np.asarray(inputs["x"], dtype=np.float32)
    Wq = np.asarray(inputs["Wq"], dtype=np.float32)
    Wk = np.asarray(inputs["Wk"], dtype=np.float32)
    Wv = np.asarray(inputs["Wv"], dtype=np.float32)
    Wo = np.asarray(inputs["Wo"], dtype=np.float32)
    B, S, _ = x.shape
    in_maps = host_inputs(x, Wq, Wk, Wv, Wo, S=S)
    if "nc" not in _cached:
        _cached["nc"] = build_nc(S=S)
    res = run_bass_kernel_spmd(_cached["nc"], in_maps, list(range(NCORES)))
    out = np.zeros((B, S, D), np.float64)
    for c, r in enumerate(res.results):
        out[c // 4] += np.asarray(r["o"], np.float32)
    return out.astype(np.float32)


# revision 9
# speedup vs baseline: 1.4809x; 1.0059x over previous
"""GQA (32q/8kv heads, RoPE, causal) TRN2 kernel v3.

Sharding: 8 cores = 2 batches x 4 kv-pairs. Core (b, kvp) owns batch b,
kv heads {2kvp, 2kvp+1}, q heads 8kvp..8kvp+7. Each core emits a partial
o [S, D] (bf16); host sums 4 partials per batch.

Precision: QKV projection in fp8e4 DoubleRow (0.5 cyc/row; contraction 2048
averages the quantization noise away). Attention core in bf16 (fp8 scores/
probs/V/at each cost 2-3% output error — the attention output is ~1/sqrt(n)
smaller than V so quantization noise does NOT average down relative to it).

Per-core pipeline:
  QKV proj (fp8 DR, token-layout) -> psum [128t, 1024]
  RoPE: 3 DVE ops (pair-swap with signed-sin table) -> qk bf16 [128, 640]
  PE transposes (bf16) -> qt [64, 8, S] / kt [64, 2, S]; V -> vsb (+ones col)
  Scores (bf16, trimmed to [qlo,512)): psc [128k, 512q]; diagonal gets a
    -240 triangular tile added via a second matmul into the same psum group
  exp on ACT (trimmed) -> probs bf16 [128, kb, 512]; Pool memsets [0,qlo)
  AV orientation-2: out = attn [128 tok, 65]: lhsT = probs block [128k, 128t]
    stationary, rhs = vsb [128k, 65] moving (65 free = full PE util); 4-head
    slabs [128, 4, 65] per psum bank; col 64 = denominator (per-partition!)
  Normalize: DVE recip [128,4] + one broadcast-free mul -> atq bf16 [128, 512]
  at transpose (PE) -> atT [128 chan, 4, S]; o-proj bf16 -> po [128, 512];
  Pool evicts to bf16, DMA out per token tile.
"""
import numpy as np
from contextlib import ExitStack

import concourse.bass as bass
from concourse import bacc
import concourse.mybir as mybir
import concourse.tile as tile
from concourse.bass_utils import run_bass_kernel_spmd
import ml_dtypes

F32 = mybir.dt.float32
BF16 = mybir.dt.bfloat16
FP8 = mybir.dt.float8e4
EXP = mybir.ActivationFunctionType.Exp
DR = mybir.MatmulPerfMode.DoubleRow

D = 2048
DH = 64
NCORES = 8
ROPE_BASE = 10000.0
MASKVAL = -240.0
EBIAS = -2.0     # probs = exp(0.125*scores - 2); cancels in normalization

_cached = {}


def build_nc(S=2048, dbg=False):
    NTT = S // 128
    NIT = S // 512
    KC = D // 128
    NH = 8
    nc = bacc.Bacc("TRN2", target_bir_lowering=False, debug=False)
    dbg_d = {}
    if dbg:
        dbg_d["d_qt"] = nc.declare_dram_parameter("d_qt", [64, NH, S], F32, isOutput=True)
        dbg_d["d_kt"] = nc.declare_dram_parameter("d_kt", [64, 2, S], F32, isOutput=True)
        dbg_d["d_vsb"] = nc.declare_dram_parameter("d_vsb", [128, 2, NTT, 65], F32, isOutput=True)
        dbg_d["d_at"] = nc.declare_dram_parameter("d_at", [128, 4, S], F32, isOutput=True)
        dbg_d["d_pb"] = nc.declare_dram_parameter("d_pb", [128, NTT, 512], F32, isOutput=True)
    xt8 = nc.declare_dram_parameter("xt8", [NTT, 128, KC, 2, 128], FP8, isOutput=False)
    wall8 = nc.declare_dram_parameter("wall8", [128, KC, 2, 768], FP8, isOutput=False)
    wotb = nc.declare_dram_parameter("wotb", [128, 4, D], BF16, isOutput=False)
    cosb = nc.declare_dram_parameter("cosb", [NTT, 128, 64], BF16, isOutput=False)
    sinsg = nc.declare_dram_parameter("sinsg", [NTT, 128, 64], BF16, isOutput=False)
    identf = nc.declare_dram_parameter("identf", [128, 128], F32, isOutput=False)
    trif = nc.declare_dram_parameter("trif", [128, 128], F32, isOutput=False)
    o = nc.declare_dram_parameter("o", [S, D], BF16, isOutput=True)

    with tile.TileContext(nc) as tc, ExitStack() as ctx:
        wp = ctx.enter_context(tc.tile_pool(name="weights", bufs=1))
        sp = ctx.enter_context(tc.tile_pool(name="state", bufs=1))
        xs = ctx.enter_context(tc.tile_pool(name="xstream", bufs=2))
        rp = ctx.enter_context(tc.tile_pool(name="ring", bufs=3))
        pr = ctx.enter_context(tc.tile_pool(name="probs", bufs=3))
        aq = ctx.enter_context(tc.tile_pool(name="atq", bufs=2))
        ob = ctx.enter_context(tc.tile_pool(name="osb", bufs=2))
        sm = ctx.enter_context(tc.tile_pool(name="small", bufs=2))

        # ---------- persistent weights / tables ----------
        wall = wp.tile([128, KC, 2, 768], FP8, tag="wall")
        wot = wp.tile([128, 4, D], BF16, tag="wot")
        cos_sb = wp.tile([128, NTT, 64], BF16, tag="cos")
        sin_sb = wp.tile([128, NTT, 64], BF16, tag="sin")
        idb = wp.tile([128, 128], BF16, tag="idb")
        trib = wp.tile([128, 128], BF16, tag="trib")
        nbias = wp.tile([128, 1], F32, tag="nbias")

        nc.sync.dma_start(wall[:], wall8[:, :, :, :])
        nc.sync.dma_start(wot[:], wotb[:, :, :])
        nc.sync.dma_start(cos_sb[:], cosb[:, :, :].rearrange("tt p c -> p tt c"))
        nc.sync.dma_start(sin_sb[:], sinsg[:, :, :].rearrange("tt p c -> p tt c"))
        idf_s = sm.tile([128, 128], F32, tag="idf")
        trf_s = sm.tile([128, 128], F32, tag="trf")
        nc.sync.dma_start(idf_s[:], identf[:, :])
        nc.sync.dma_start(trf_s[:], trif[:, :])
        nc.vector.tensor_copy(idb[:], idf_s[:])
        nc.vector.tensor_copy(trib[:], trf_s[:])
        nc.vector.memset(nbias[:], EBIAS)

        # ---------- per-core state ----------
        qt = sp.tile([64, NH, S], BF16, tag="qt")
        kt = sp.tile([64, 2, S], BF16, tag="kt")
        vsb = sp.tile([128, 2, NTT, 65], BF16, tag="vsb")
        atT = sp.tile([128, 4, S], BF16, tag="atT")
        nc.vector.memset(vsb[:, :, :, 64:65], 1.0)

        # ================= phase 1: QKV + rope + transposes =================
        pq_pool = ExitStack()
        pp_qkv = pq_pool.enter_context(tc.tile_pool(name="pqkv", bufs=2, space="PSUM"))
        pp_tr = pq_pool.enter_context(tc.tile_pool(name="ptr", bufs=2, space="PSUM"))
        pre_pool = ExitStack()
        pp_pre = pre_pool.enter_context(tc.tile_pool(name="presc", bufs=1, space="PSUM"))

        pbs = [None] * NH
        prescored = set()

        def emit_scores(it, h, pool, scbufs, drain=None):
            kv = h // 4
            nkb = 4 * it + 4
            i0 = it * 512
            tag = "probs0" if (it == 0 and NTT >= 16) else "probs"
            pbufs = 8 if tag == "probs0" else 2
            pb = pr.tile([128, nkb, 512], BF16, tag=tag, bufs=pbufs,
                         name=f"pb{it}_{h}")
            pbs[h] = pb
            for kb in range(nkb):
                if drain and kb and kb % (max(2, nkb // 2)) == 0:
                    drain(1)
                diag = kb >= 4 * it
                qlo = (kb - 4 * it) * 128 if diag else 0
                psc = pool.tile([128, 512], F32, tag="sc", bufs=scbufs,
                                name=f"psc{it}_{h}_{kb}")
                nc.tensor.matmul(psc[:, qlo:512],
                                 kt[:, kv, kb * 128:(kb + 1) * 128],
                                 qt[:, h, i0 + qlo:i0 + 512],
                                 start=True, stop=not diag)
                if diag:
                    nc.tensor.matmul(psc[:, qlo:qlo + 128], idb[:], trib[:],
                                     start=False, stop=True)
                nc.scalar.activation(pb[:, kb, qlo:512], psc[:, qlo:512],
                                     EXP, scale=0.125, bias=nbias[:])
                if qlo:
                    nc.gpsimd.memset(pb[:, kb, 0:qlo], 0.0)

        def p1_tail(tt, qk8):
            tsl = slice(tt * 128, (tt + 1) * 128)
            qtr = pp_tr.tile([64, 8, 128], BF16, tag="qtr", name=f"qtr{tt}")
            ktr = pp_tr.tile([64, 8, 128], BF16, tag="qtr", name=f"ktr{tt}")
            ktr = ktr[:, 0:2, :]
            for h in range(8):
                nc.tensor.matmul(qtr[:, h, :], qk8[:, h * 64:(h + 1) * 64],
                                 idb[:], is_transpose=True,
                                 start=(h == 0), stop=(h == 7))
            for g in range(2):
                nc.tensor.matmul(ktr[:, g, :],
                                 qk8[:, 512 + g * 64:512 + (g + 1) * 64],
                                 idb[:], is_transpose=True,
                                 start=(g == 0), stop=(g == 1))
            nc.vector.tensor_copy(qt[:, :, tsl], qtr[:])
            nc.vector.tensor_copy(kt[:, :, tsl], ktr[:])

        prevq = []
        for tt in range(NTT):
            xtile = xs.tile([128, KC, 2, 128], FP8, tag="xt", name=f"xt{tt}")
            nc.sync.dma_start(xtile[:], xt8[tt])
            pq = pp_qkv.tile([128, 1024], F32, tag="pq", name=f"pq{tt}")
            # exact-ish fp8 hi/lo split: (xh+xl)(wh+wl) ~ xh wh + xl wh + xh wl
            # per chunk-pair: 3 DoubleRow matmuls (0.75x bf16 cost); xl*wl dropped
            for kp in range(KC // 2):
                xh = xtile[:, 2 * kp:2 * kp + 2, 0, :]
                xl = xtile[:, 2 * kp:2 * kp + 2, 1, :]
                for c0, c1 in ((0, 512), (512, 768)):
                    wh = wall[:, 2 * kp:2 * kp + 2, 0, c0:c1]
                    wl = wall[:, 2 * kp:2 * kp + 2, 1, c0:c1]
                    st = (kp == 0)
                    nc.tensor.matmul(pq[:, c0:c1], xh, wh, start=st, stop=False,
                                     perf_mode=DR)
                    nc.tensor.matmul(pq[:, c0:c1], xl, wh, start=False, stop=False,
                                     perf_mode=DR)
                    nc.tensor.matmul(pq[:, c0:c1], xh, wl, start=False,
                                     stop=(kp == KC // 2 - 1), perf_mode=DR)
            if len(prevq) >= 2:
                p1_tail(*prevq.pop(0))
            if 5 <= tt < 13 and NTT >= 16:
                emit_scores(0, tt - 5, pp_pre, 2)
                prescored.add(tt - 5)
            # ACT evicts psum -> bf16 sbuf (GPSIMD cannot touch PSUM);
            # rope: op1 tmp = pairswap(qk)*sinsg (Pool); op2 t1 = qk*cos (Pool);
            # op3 qk8 = t1+tmp (DVE, bf16 2x)
            qkvb = rp.tile([128, 768], BF16, tag="qkvb")
            nc.vector.tensor_scalar_mul(qkvb[:], pq[:, 0:768], 1.0 / 64.0)
            qkv = qkvb[:, 0:640]
            swp = qkv.rearrange("p (h n two) -> p h n two", two=2, n=32)[..., ::-1]
            tmp = rp.tile([128, 640], BF16, tag="tmp")
            t1 = rp.tile([128, 640], BF16, tag="t1")
            qk8 = rp.tile([128, 640], BF16, tag="qk8")
            sin4 = sin_sb[:, tt, :].rearrange("p (one n two) -> p one n two",
                                              one=1, two=2).to_broadcast([128, 10, 32, 2])
            cos3 = cos_sb[:, tt, :].rearrange("p (one c) -> p one c",
                                              one=1).to_broadcast([128, 10, 64])
            nc.gpsimd.tensor_mul(tmp[:].rearrange("p (h n two) -> p h n two",
                                                  two=2, n=32),
                                 swp, sin4)
            nc.gpsimd.tensor_mul(t1[:].rearrange("p (h c) -> p h c", h=10),
                                 qkv.rearrange("p (h c) -> p h c", h=10), cos3)
            nc.vector.tensor_add(qk8[:], t1[:], tmp[:])
            nc.vector.tensor_copy(vsb[:, :, tt, 0:64],
                                  qkvb[:, 640:768].rearrange("p (kv c) -> p kv c",
                                                             kv=2))
            prevq.append((tt, qk8))
        for pv in prevq:
            p1_tail(*pv)
        pre_pool.close()
        pq_pool.close()

        # ================= phase 2+3: attention + o-proj =================
        pp_att = ctx.enter_context(tc.tile_pool(name="patt", bufs=1, space="PSUM"))
        avs = [None] * 2
        oproj_q = []

        for it in range(NIT):
            i0 = it * 512

            def emit_av(h):
                kv = h // 4
                pb = pbs[h]
                for tq in range(4):
                    tt = 4 * it + tq
                    if h % 2 == 0 and tq % 2 == 0:
                        avs[tq // 2] = pp_att.tile([128, 2, 2, 128], F32,
                                                   tag=f"avs{tq // 2}", bufs=1,
                                                   name=f"avs{it}_{h}_{tq}")
                    slab = avs[tq // 2]
                    for kb in range(tt + 1):
                        nc.tensor.matmul(slab[:, tq % 2, h % 2, 0:65],
                                         pb[:, kb, tq * 128:(tq + 1) * 128],
                                         vsb[:, kv, kb, :],
                                         start=(kb == 0), stop=(kb == tt),
                                         skip_group_check=True)

            def emit_norm(g):
                for tq in range(4):
                    tt = 4 * it + tq
                    tsl = slice(tt * 128, (tt + 1) * 128)
                    slab = avs[tq // 2][:, tq % 2]
                    rec = sm.tile([128, 2, 1], F32, tag="rec")
                    nc.vector.reciprocal(rec[:, :, 0], slab[:, :, 64])
                    atq = aq.tile([128, 2, 64], BF16, tag="atq",
                                  name=f"atq{it}_{g}_{tq}")
                    nc.vector.tensor_mul(atq[:], slab[:, :, 0:64],
                                         rec[:].to_broadcast([128, 2, 64]))
                    attr = pp_att.tile([128, 128], BF16, tag="sc", bufs=4,
                                       name=f"attr{it}_{g}_{tq}")
                    nc.tensor.matmul(attr[:],
                                     atq[:].rearrange("p f c -> p (f c)"),
                                     idb[:], is_transpose=True,
                                     start=True, stop=True)
                    nc.vector.tensor_copy(atT[:, g, tsl], attr[:])

            def drain_oproj(n):
                for _ in range(n):
                    if oproj_q:
                        oproj_q.pop(0)()

            def maybe_scores(h):
                if it == 0 and h in prescored:
                    return
                emit_scores(it, h, pp_att, 4, drain=drain_oproj)

            maybe_scores(0)
            drain_oproj(2)
            for h in range(1, NH):
                maybe_scores(h)
                emit_av(h - 1)
                if h % 2 == 0:
                    emit_norm(h // 2 - 1)
                drain_oproj(2)
            emit_av(NH - 1)
            emit_norm(NH // 2 - 1)

            def queue_oproj(it_):
                state = {}
                for tq in range(4):
                    tt = 4 * it_ + tq
                    for nt in range(4):
                        def step(tt=tt, nt=nt):
                            tsl = slice(tt * 128, (tt + 1) * 128)
                            if nt == 0:
                                state[tt] = ob.tile([128, D], BF16, tag="osb",
                                                    name=f"osb{tt}")
                            osb = state[tt]
                            nsl = slice(nt * 512, (nt + 1) * 512)
                            po = pp_att.tile([128, 512], F32, tag="po", bufs=2,
                                             name=f"po{tt}_{nt}")
                            for c in range(4):
                                nc.tensor.matmul(po[:], atT[:, c, tsl],
                                                 wot[:, c, nsl],
                                                 start=(c == 0), stop=(c == 3))
                            nc.vector.tensor_copy(osb[:, nsl], po[:])
                            if nt == 3:
                                nc.sync.dma_start(o[tsl, :], osb[:])
                        oproj_q.append(step)
            queue_oproj(it)
        while oproj_q:
            oproj_q.pop(0)()

        if dbg:
            dsc = ctx.enter_context(tc.tile_pool(name="dsc", bufs=1))
            for nm, t in [("d_qt", qt[:]), ("d_kt", kt[:]), ("d_vsb", vsb[:]),
                          ("d_at", atT[:]), ("d_pb", pbs[0][:])]:
                f = dsc.tile(list(t.shape), F32, tag="f" + nm, name="f" + nm)
                nc.vector.tensor_copy(f[:], t)
                nc.sync.dma_start(dbg_d[nm][tuple(slice(None) for _ in t.shape)], f[:])
    nc.compile()
    return nc


# ====================== host side ======================

def _fp8(x):
    return np.asarray(x, np.float32).astype(ml_dtypes.float8_e4m3)


def host_inputs(x, Wq, Wk, Wv, Wo, S=2048):
    NTT = S // 128
    KC = D // 128
    inv = ROPE_BASE ** (-np.arange(0, DH, 2, dtype=np.float64) / DH)
    th = np.arange(S, dtype=np.float64)[:, None] * inv[None, :]
    cos1 = np.repeat(np.cos(th), 2, axis=1)
    sin1 = np.sin(th)
    sinsg1 = np.empty((S, 64))
    sinsg1[:, 0::2] = -sin1
    sinsg1[:, 1::2] = sin1
    cosb = cos1.reshape(NTT, 128, 64).astype(ml_dtypes.bfloat16)
    sing = sinsg1.reshape(NTT, 128, 64).astype(ml_dtypes.bfloat16)
    identf = np.eye(128, dtype=np.float32)
    p = np.arange(128)[:, None]
    q = np.arange(128)[None, :]
    trif = np.where(p <= q, 0.0, MASKVAL).astype(np.float32)

    in_maps = []
    for c in range(NCORES):
        b, kvp = c // 4, c % 4
        xb = np.asarray(x[b], np.float32)
        xh = _fp8(xb)
        xl = _fp8(xb - xh.astype(np.float32))
        xt = np.stack([xh, xl], axis=0).reshape(2, NTT, 128, KC, 128)
        xt8 = np.ascontiguousarray(xt.transpose(1, 4, 3, 0, 2))
        wq = Wq[512 * kvp:512 * (kvp + 1)]
        wk = Wk[128 * kvp:128 * (kvp + 1)]
        wv = Wv[128 * kvp:128 * (kvp + 1)]
        wall = np.concatenate([wq, wk, wv], axis=0) * 64.0
        wh = _fp8(wall)
        wl = _fp8(wall - wh.astype(np.float32))
        wall8 = np.ascontiguousarray(
            np.stack([wh, wl], axis=0).transpose(2, 0, 1)
            .reshape(KC, 128, 2, 768).transpose(1, 0, 2, 3))
        wotb = np.ascontiguousarray(
            Wo[:, 512 * kvp:512 * (kvp + 1)].astype(ml_dtypes.bfloat16)
            .T.reshape(4, 128, D).transpose(1, 0, 2))
        in_maps.append(dict(xt8=xt8, wall8=wall8, wotb=wotb, cosb=cosb,
                            sinsg=sing, identf=identf, trif=trif))
    return in_maps


def kernel(**inputs):
    x = np.asarray(inputs["x"], dtype=np.float32)
    Wq = np.asarray(inputs["Wq"], dtype=np.float32)
    Wk = np.asarray(inputs["Wk"], dtype=np.float32)
    Wv = np.asarray(inputs["Wv"], dtype=np.float32)
    Wo = np.asarray(inputs["Wo"], dtype=np.float32)
    B, S, _ = x.shape
    in_maps = host_inputs(x, Wq, Wk, Wv, Wo, S=S)
    if "nc" not in _cached:
        _cached["nc"] = build_nc(S=S)
    res = run_bass_kernel_spmd(_cached["nc"], in_maps, list(range(NCORES)))
    out = np.zeros((B, S, D), np.float64)
    for c, r in enumerate(res.results):
        out[c // 4] += np.asarray(r["o"], np.float32)
    return out.astype(np.float32)
